# revision 1
# baseline (speedup 1.0000x reference)
"""Trainium2 Bass kernel for a 4D ConvBlock (conv3^4 -> LN -> GELU -> 1x1 conv -> residual).

Strategy (8 NeuronCores, data-parallel over T with halo 1):
  - Core t computes the full output t-slice out[:, :, t] for BOTH batch samples.
  - Partition layout: 128 SBUF partitions = (sample n)*64 + channel c.
  - conv1 is computed as 81 accumulating PE matmuls (one per 3x3x3x3 kernel
    offset) with BLOCK-DIAGONAL weights [128,128] so both samples ride one
    matmul (K=64 channels would otherwise waste half the 128-wide PE array).
  - Spatial H/W halos come from zero-padded SBUF slices (34x34 per (l) slice,
    padded on host); L halos are handled by skipping out-of-range dl offsets;
    T halos by zero-filled neighbor t-slices on edge cores.
  - Channel-wise LayerNorm stats via tiny matmuls (ones-reduce K=128->M=2 per
    sample), broadcast back with a [2->128] matmul; exact-erf GELU on ACT.
  - conv2 (1x1) is a single block-diagonal matmul; residual read straight from
    the padded input slice.
  - Matmuls run in float32r (TF32, full PE rate). The BIR verifier requires
    every matmul operand's producer to round to f32r, so matmul-feeding tiles
    are DECLARED float32r (DMA'd ones come from f32r DRAM tensors; computed
    ones are written by ACT/DVE ops that round on write). Non-matmul consumers
    read those tiles through a bitcast back to f32.
"""
import os
import sys

os.environ.setdefault("MYCRO_LOCAL_CACHE", "1")
for _p in ("/opt/trn_rl_repo",):
    if os.path.isdir(_p) and _p not in sys.path:
        sys.path.insert(0, _p)

import numpy as np

import concourse.bass as bass
import concourse.tile as tile
from concourse import bacc, mybir
from concourse import bass_utils

# float32 = exact, quarter-rate PE. float32r = TF32, full-rate PE.
MM_DTYPE = os.environ.get("MM_DTYPE", "float32r")
TRACE = os.environ.get("KERNEL_TRACE", "0") == "1"

N, C, T, L, H, W = 2, 64, 8, 8, 32, 32
P = 128
EPS = 1e-5
OFFSETS = [(dt, dl, dh, dw)
           for dt in (-1, 0, 1) for dl in (-1, 0, 1)
           for dh in (-1, 0, 1) for dw in (-1, 0, 1)]

_CACHE = {}
LAST_RESULTS = None


def _build(mm_dtype_str):
    f32 = mybir.dt.float32
    mmdt = getattr(mybir.dt, mm_dtype_str)
    AF = mybir.ActivationFunctionType

    def asf32(ap):
        return ap if ap.dtype == f32 else ap.bitcast(f32)

    nc = bacc.Bacc("TRN2", target_bir_lowering=False, debug=False,
                   enable_asserts=False, num_devices=8)
    xinp = nc.dram_tensor("xinp", [3, P, L, H + 2, W + 2], mmdt,
                          kind="ExternalInput").ap()
    w1c = nc.dram_tensor("w1c", [C, 81, C], mmdt, kind="ExternalInput").ap()
    w2bd = nc.dram_tensor("w2bd", [P, P], mmdt, kind="ExternalInput").ap()
    onesbc = nc.dram_tensor("onesbc", [P, P], mmdt, kind="ExternalInput").ap()
    params = nc.dram_tensor("params", [P, 5], f32, kind="ExternalInput").ap()
    out = nc.dram_tensor("out", [P, L, H, W], f32, kind="ExternalOutput").ap()

    with tile.TileContext(nc) as tc:
        with (
            tc.tile_pool(name="wpool", bufs=1) as wpool,
            tc.tile_pool(name="xpool", bufs=4) as xpool,
            tc.tile_pool(name="work", bufs=2) as work,
            tc.tile_pool(name="ps_acc", bufs=4, space=bass.MemorySpace.PSUM) as ps_acc,
            tc.tile_pool(name="ps_bc", bufs=1, space=bass.MemorySpace.PSUM) as ps_bc,
            tc.tile_pool(name="ps_out", bufs=2, space=bass.MemorySpace.PSUM) as ps_out,
        ):
            w1sb = []

            def emit_chunk(j):
                # Emission order = DMA queue priority: chunk j is emitted
                # right before its first consuming matmul so startup queues
                # drain the truly critical bytes first.
                assert j == len(w1sb)
                w1j = wpool.tile([P, 27, P], mmdt, name=f"w1sb{j}", tag=f"w1sb{j}")
                nc.vector.memset(w1j[0:C, :, C:P].bitcast(f32), 0.0)
                nc.vector.memset(w1j[C:P, :, 0:C].bitcast(f32), 0.0)
                nc.sync.dma_start(w1j[0:C, :, 0:C],
                                  w1c[:, 27 * j: 27 * (j + 1), :])
                nc.sync.dma_start(w1j[C:P, :, C:P],
                                  w1c[:, 27 * j: 27 * (j + 1), :])
                w1sb.append(w1j)

            xs = {}

            def load_one(tb, l):
                xt = xpool.tile([P, H + 2, W + 2], mmdt,
                                name=f"x{tb}_{l}", tag=f"x{tb}")
                # two DMAs per slice -> more queues active during startup
                nc.sync.dma_start(xt[:, 0:17, :], xinp[tb, :, l, 0:17, :])
                nc.sync.dma_start(xt[:, 17:34, :], xinp[tb, :, l, 17:34, :])
                xs[(tb, l)] = xt

            def load_slice(l):
                for tb in range(3):
                    load_one(tb, l)

            def process(l):
                act_os = [o for o, (dt, dl, dh, dw) in enumerate(OFFSETS)
                          if 0 <= l + dl < L]
                act_insts = []
                for half in range(2):
                    h0 = 16 * half
                    acc = ps_acc.tile([P, 16, W], f32,
                                      name=f"acc_{l}_{half}", tag="acc")
                    for i, o in enumerate(act_os):
                        dt, dl, dh, dw = OFFSETS[o]
                        while o // 27 >= len(w1sb):
                            emit_chunk(len(w1sb))
                        rhs = xs[(dt + 1, l + dl)][:, h0 + dh + 1: h0 + dh + 17,
                                                   dw + 1: dw + 33]
                        nc.tensor.matmul(acc[:], w1sb[o // 27][:, o % 27, :], rhs,
                                         start=(i == 0),
                                         stop=(i == len(act_os) - 1))
                    h = work.tile([P, 16, W], mmdt, name=f"h_{l}_{half}", tag="h")
                    nc.vector.tensor_scalar_add(h[:], acc[:], b1_ap)
                    sq = work.tile([P, 16, W], mmdt, name=f"sq_{l}_{half}", tag="sq")
                    nc.vector.tensor_mul(sq[:], asf32(h[:]), asf32(h[:]))
                    bc_mu = ps_bc.tile([P, 16, W], f32,
                                       name=f"bcmu_{l}_{half}", tag="bc_mu")
                    nc.tensor.matmul(bc_mu[:], onsb[:], h[:])
                    bc_e2 = ps_bc.tile([P, 16, W], f32,
                                       name=f"bce2_{l}_{half}", tag="bc_e2")
                    nc.tensor.matmul(bc_e2[:], onsb[:], sq[:])
                    mu_sbf = work.tile([P, 16, W], f32,
                                       name=f"musbf_{l}_{half}", tag="mu_sbf")
                    nc.vector.tensor_copy(mu_sbf[:], bc_mu[:])
                    mu2 = work.tile([P, 16, W], f32,
                                    name=f"mu2_{l}_{half}", tag="mu2")
                    nc.vector.tensor_mul(mu2[:], mu_sbf[:], mu_sbf[:])
                    var = work.tile([P, 16, W], f32,
                                    name=f"var_{l}_{half}", tag="var")
                    nc.vector.tensor_sub(var[:], bc_e2[:], mu2[:])
                    rstd = work.tile([P, 16, W], f32,
                                     name=f"rstd_{l}_{half}", tag="rstd")
                    absr_i = nc.scalar.activation(rstd[:], var[:],
                                                  AF.Abs_reciprocal_sqrt,
                                                  bias=eps_ap, scale=1.0)
                    t1 = work.tile([P, 16, W], f32, name=f"t1_{l}_{half}", tag="t1")
                    nc.vector.tensor_sub(t1[:], asf32(h[:]), mu_sbf[:])
                    t2 = work.tile([P, 16, W], f32, name=f"t2_{l}_{half}", tag="t2")
                    nc.vector.tensor_mul(t2[:], t1[:], rstd[:])
                    g = work.tile([P, 16, W], mmdt, name=f"g_{l}_{half}", tag="g")
                    gelu_i = nc.scalar.activation(g[:], t2[:], AF.Gelu,
                                                  bias=lnb_ap, scale=lnw_ap)
                    act_insts.append((absr_i, gelu_i))
                    ps2 = ps_out.tile([P, 16, W], f32,
                                      name=f"ps2_{l}_{half}", tag="ps2")
                    nc.tensor.matmul(ps2[:], w2sb[:], g[:])
                    o1 = work.tile([P, 16, W], f32, name=f"o1_{l}_{half}", tag="o1")
                    nc.vector.tensor_scalar_add(o1[:], ps2[:], b2_ap)
                    osb = work.tile([P, 16, W], f32,
                                    name=f"osb_{l}_{half}", tag="osb")
                    nc.vector.tensor_add(osb[:], o1[:],
                                         asf32(xs[(1, l)][:, h0 + 1: h0 + 17, 1: 33]))
                    nc.sync.dma_start(out[:, l, h0: h0 + 16, :], osb[:])
                if len(act_insts) == 2:
                    tile.add_dep_helper(
                        act_insts[0][1].ins, act_insts[1][0].ins, sync=True,
                        reason="batch ACT funcs: absr0,absr1,gelu0,gelu1")

            # Emission order == queue-FIFO priority == matmul consumption
            # order: chunk0, then slices tb-major (dt=-1 block reads xp first).
            emit_chunk(0)
            for _tb in range(3):
                load_one(_tb, 0)
                load_one(_tb, 1)
            w2sb = wpool.tile([P, P], mmdt, name="w2sb", tag="w2sb")
            nc.sync.dma_start(w2sb[:], w2bd[:])
            onsb = wpool.tile([P, P], mmdt, name="onsb", tag="onsb")
            nc.sync.dma_start(onsb[:], onesbc[:])
            psb = wpool.tile([P, 5], f32, name="psb", tag="psb")
            nc.sync.dma_start(psb[:], params[:])
            b1_ap = psb[:, 0:1]
            lnw_ap = psb[:, 1:2]
            lnb_ap = psb[:, 2:3]
            b2_ap = psb[:, 3:4]
            eps_ap = psb[:, 4:5]

            process(0)
            for l in range(2, L + 1):
                if l < L:
                    load_slice(l)
                process(l - 1)

    nc.compile()
    return nc


def _get_program():
    key = MM_DTYPE
    if key not in _CACHE:
        _CACHE[key] = _build(key)
    return _CACHE[key]


def _host_prep(x, w1, b1, ln_w, ln_b, w2, b2):
    x = np.ascontiguousarray(np.asarray(x, dtype=np.float32))
    xm = x.reshape(N * C, T, L, H, W)
    # pad H and W by 1 on each side with zeros
    xpad = np.zeros((N * C, T, L, H + 2, W + 2), np.float32)
    xpad[:, :, :, 1:H + 1, 1:W + 1] = xm
    zslice = np.zeros((N * C, L, H + 2, W + 2), np.float32)
    xins = []
    for t in range(T):
        xp = xpad[:, t - 1] if t > 0 else zslice
        xc = xpad[:, t]
        xn = xpad[:, t + 1] if t < T - 1 else zslice
        xins.append(np.ascontiguousarray(np.stack([xp, xc, xn])))

    w1c = np.ascontiguousarray(
        np.asarray(w1, dtype=np.float32).transpose(1, 2, 3, 4, 5, 0)
    ).reshape(C, 81, C)
    w2t = np.asarray(w2, dtype=np.float32).reshape(C, C).T
    w2bd = np.zeros((P, P), np.float32)
    w2bd[:C, :C] = w2t
    w2bd[C:, C:] = w2t
    onesbc = np.zeros((P, P), np.float32)
    onesbc[:C, :C] = 1.0 / C
    onesbc[C:, C:] = 1.0 / C
    params = np.zeros((P, 5), np.float32)
    params[:, 0] = np.tile(np.asarray(b1, dtype=np.float32), 2)
    params[:, 1] = np.tile(np.asarray(ln_w, dtype=np.float32), 2)
    params[:, 2] = np.tile(np.asarray(ln_b, dtype=np.float32), 2)
    params[:, 3] = np.tile(np.asarray(b2, dtype=np.float32), 2)
    params[:, 4] = EPS
    return xins, w1c, w2bd, onesbc, params


def kernel(x, w1, b1, ln_w, ln_b, w2, b2):
    global LAST_RESULTS
    xins, w1c, w2bd, onesbc, params = _host_prep(
        x, w1, b1, ln_w, ln_b, w2, b2)
    nc = _get_program()
    in_maps = [
        {"xinp": xins[t], "w1c": w1c, "w2bd": w2bd, "onesbc": onesbc,
         "params": params}
        for t in range(T)
    ]
    res = bass_utils.run_bass_kernel_spmd(
        nc, in_maps, core_ids=list(range(8)), trace=TRACE)
    LAST_RESULTS = res
    out = np.stack([res.results[t]["out"] for t in range(T)], axis=1)
    return np.ascontiguousarray(out.reshape(N, C, T, L, H, W))



# revision 4
# speedup vs baseline: 1.0977x; 1.0977x over previous
"""Trainium2 Bass kernel for a 4D ConvBlock (conv3^4 -> LN -> GELU -> 1x1 conv -> residual).

Strategy (8 NeuronCores, data-parallel over T with halo 1):
  - Core t computes the full output t-slice out[:, :, t] for BOTH batch samples.
  - Partition layout: 128 SBUF partitions = (sample n)*64 + channel c.
  - conv1 is computed as 81 accumulating PE matmuls (one per 3x3x3x3 kernel
    offset) with BLOCK-DIAGONAL weights [128,128] so both samples ride one
    matmul (K=64 channels would otherwise waste half the 128-wide PE array).
  - Spatial H/W halos come from zero-padded SBUF slices (34x34 per (l) slice,
    padded on host); L halos are handled by skipping out-of-range dl offsets;
    T halos by zero-filled neighbor t-slices on edge cores.
  - Channel-wise LayerNorm stats via tiny matmuls (ones-reduce K=128->M=2 per
    sample), broadcast back with a [2->128] matmul; exact-erf GELU on ACT.
  - conv2 (1x1) is a single block-diagonal matmul; residual read straight from
    the padded input slice.
  - Matmuls run in float32r (TF32, full PE rate). The BIR verifier requires
    every matmul operand's producer to round to f32r, so matmul-feeding tiles
    are DECLARED float32r (DMA'd ones come from f32r DRAM tensors; computed
    ones are written by ACT/DVE ops that round on write). Non-matmul consumers
    read those tiles through a bitcast back to f32.
"""
import os
import sys

os.environ.setdefault("MYCRO_LOCAL_CACHE", "1")
for _p in ("/opt/trn_rl_repo",):
    if os.path.isdir(_p) and _p not in sys.path:
        sys.path.insert(0, _p)

import numpy as np

import concourse.bass as bass
import concourse.tile as tile
from concourse import bacc, mybir
from concourse import bass_utils

# float32 = exact, quarter-rate PE. float32r = TF32, full-rate PE.
MM_DTYPE = os.environ.get("MM_DTYPE", "float32r")
TRACE = os.environ.get("KERNEL_TRACE", "0") == "1"

N, C, T, L, H, W = 2, 64, 8, 8, 32, 32
P = 128
EPS = 1e-5
OFFSETS = [(dt, dl, dh, dw)
           for dt in (-1, 0, 1) for dl in (-1, 0, 1)
           for dh in (-1, 0, 1) for dw in (-1, 0, 1)]

_CACHE = {}
LAST_RESULTS = None


def _build(mm_dtype_str):
    f32 = mybir.dt.float32
    mmdt = getattr(mybir.dt, mm_dtype_str)
    AF = mybir.ActivationFunctionType

    def asf32(ap):
        if ap.dtype == f32:
            return ap
        if mybir.dt.size(ap.dtype) == 4:
            return ap.bitcast(f32)
        return ap  # 16-bit dtypes: engines convert on read

    nc = bacc.Bacc("TRN2", target_bir_lowering=False, debug=False,
                   enable_asserts=False, num_devices=8)
    xinp = nc.dram_tensor("xinp", [3, P, L, H + 2, W + 2], mmdt,
                          kind="ExternalInput").ap()
    w1c = nc.dram_tensor("w1c", [C, 81, C], mmdt, kind="ExternalInput").ap()
    w2bd = nc.dram_tensor("w2bd", [P, P], mmdt, kind="ExternalInput").ap()
    onesbc = nc.dram_tensor("onesbc", [P, P], mmdt, kind="ExternalInput").ap()
    params = nc.dram_tensor("params", [P, 5], f32, kind="ExternalInput").ap()
    out = nc.dram_tensor("out", [P, L, H, W], f32, kind="ExternalOutput").ap()

    with tile.TileContext(nc) as tc:
        with (
            tc.tile_pool(name="wpool", bufs=1) as wpool,
            tc.tile_pool(name="xpool", bufs=4) as xpool,
            tc.tile_pool(name="work", bufs=2) as work,
            tc.tile_pool(name="ps_acc", bufs=4, space=bass.MemorySpace.PSUM) as ps_acc,
            tc.tile_pool(name="ps_bc", bufs=1, space=bass.MemorySpace.PSUM) as ps_bc,
            tc.tile_pool(name="ps_out", bufs=2, space=bass.MemorySpace.PSUM) as ps_out,
        ):
            w1sb = []

            def emit_chunk(j):
                # Emission order = DMA queue priority: chunk j is emitted
                # right before its first consuming matmul so startup queues
                # drain the truly critical bytes first.
                assert j == len(w1sb)
                w1j = wpool.tile([P, 27, P], mmdt, name=f"w1sb{j}", tag=f"w1sb{j}")
                zmm = (lambda ap: ap.bitcast(f32)) if mybir.dt.size(mmdt) == 4 \
                    else (lambda ap: ap)
                nc.vector.memset(zmm(w1j[0:C, :, C:P]), 0.0)
                nc.vector.memset(zmm(w1j[C:P, :, 0:C]), 0.0)
                nc.sync.dma_start(w1j[0:C, :, 0:C],
                                  w1c[:, 27 * j: 27 * (j + 1), :])
                nc.sync.dma_start(w1j[C:P, :, C:P],
                                  w1c[:, 27 * j: 27 * (j + 1), :])
                w1sb.append(w1j)

            xs = {}

            def load_one(tb, l):
                xt = xpool.tile([P, H + 2, W + 2], mmdt,
                                name=f"x{tb}_{l}", tag=f"x{tb}")
                # two DMAs per slice -> more queues active during startup
                nc.sync.dma_start(xt[:, 0:17, :], xinp[tb, :, l, 0:17, :])
                nc.sync.dma_start(xt[:, 17:34, :], xinp[tb, :, l, 17:34, :])
                xs[(tb, l)] = xt

            def load_slice(l):
                for tb in range(3):
                    load_one(tb, l)

            def process(l):
                act_os = [o for o, (dt, dl, dh, dw) in enumerate(OFFSETS)
                          if 0 <= l + dl < L]
                act_insts = []
                for half in range(2):
                    h0 = 16 * half
                    acc = ps_acc.tile([P, 16, W], f32,
                                      name=f"acc_{l}_{half}", tag="acc")
                    for i, o in enumerate(act_os):
                        dt, dl, dh, dw = OFFSETS[o]
                        while o // 27 >= len(w1sb):
                            emit_chunk(len(w1sb))
                        rhs = xs[(dt + 1, l + dl)][:, h0 + dh + 1: h0 + dh + 17,
                                                   dw + 1: dw + 33]
                        nc.tensor.matmul(acc[:], w1sb[o // 27][:, o % 27, :], rhs,
                                         start=(i == 0),
                                         stop=(i == len(act_os) - 1))
                    h = work.tile([P, 16, W], mmdt, name=f"h_{l}_{half}", tag="h")
                    nc.vector.tensor_scalar_add(h[:], acc[:], b1_ap)
                    sq = work.tile([P, 16, W], mmdt, name=f"sq_{l}_{half}", tag="sq")
                    nc.vector.tensor_mul(sq[:], asf32(h[:]), asf32(h[:]))
                    bc_mu = ps_bc.tile([P, 16, W], f32,
                                       name=f"bcmu_{l}_{half}", tag="bc_mu")
                    nc.tensor.matmul(bc_mu[:], onsb[:], h[:])
                    bc_e2 = ps_bc.tile([P, 16, W], f32,
                                       name=f"bce2_{l}_{half}", tag="bc_e2")
                    nc.tensor.matmul(bc_e2[:], onsb[:], sq[:])
                    mu_sbf = work.tile([P, 16, W], f32,
                                       name=f"musbf_{l}_{half}", tag="mu_sbf")
                    nc.vector.tensor_copy(mu_sbf[:], bc_mu[:])
                    mu2 = work.tile([P, 16, W], f32,
                                    name=f"mu2_{l}_{half}", tag="mu2")
                    nc.vector.tensor_mul(mu2[:], mu_sbf[:], mu_sbf[:])
                    var = work.tile([P, 16, W], f32,
                                    name=f"var_{l}_{half}", tag="var")
                    nc.vector.tensor_sub(var[:], bc_e2[:], mu2[:])
                    rstd = work.tile([P, 16, W], f32,
                                     name=f"rstd_{l}_{half}", tag="rstd")
                    absr_i = nc.scalar.activation(rstd[:], var[:],
                                                  AF.Abs_reciprocal_sqrt,
                                                  bias=eps_ap, scale=1.0)
                    t1 = work.tile([P, 16, W], f32, name=f"t1_{l}_{half}", tag="t1")
                    nc.vector.tensor_sub(t1[:], asf32(h[:]), mu_sbf[:])
                    t2 = work.tile([P, 16, W], f32, name=f"t2_{l}_{half}", tag="t2")
                    nc.vector.tensor_mul(t2[:], t1[:], rstd[:])
                    g = work.tile([P, 16, W], mmdt, name=f"g_{l}_{half}", tag="g")
                    gelu_i = nc.scalar.activation(g[:], t2[:], AF.Gelu,
                                                  bias=lnb_ap, scale=lnw_ap)
                    act_insts.append((absr_i, gelu_i))
                    ps2 = ps_out.tile([P, 16, W], f32,
                                      name=f"ps2_{l}_{half}", tag="ps2")
                    nc.tensor.matmul(ps2[:], w2sb[:], g[:])
                    o1 = work.tile([P, 16, W], f32, name=f"o1_{l}_{half}", tag="o1")
                    nc.vector.tensor_scalar_add(o1[:], ps2[:], b2_ap)
                    osb = work.tile([P, 16, W], f32,
                                    name=f"osb_{l}_{half}", tag="osb")
                    nc.vector.tensor_add(osb[:], o1[:],
                                         asf32(xs[(1, l)][:, h0 + 1: h0 + 17, 1: 33]))
                    nc.sync.dma_start(out[:, l, h0: h0 + 16, :], osb[:])
                if len(act_insts) == 2:
                    tile.add_dep_helper(
                        act_insts[0][1].ins, act_insts[1][0].ins, sync=True,
                        reason="batch ACT funcs: absr0,absr1,gelu0,gelu1")

            # Emission order == queue-FIFO priority == matmul consumption
            # order: chunk0, then slices tb-major (dt=-1 block reads xp first).
            emit_chunk(0)
            for _tb in range(3):
                load_one(_tb, 0)
                load_one(_tb, 1)
            w2sb = wpool.tile([P, P], mmdt, name="w2sb", tag="w2sb")
            nc.sync.dma_start(w2sb[:], w2bd[:])
            onsb = wpool.tile([P, P], mmdt, name="onsb", tag="onsb")
            nc.sync.dma_start(onsb[:], onesbc[:])
            psb = wpool.tile([P, 5], f32, name="psb", tag="psb")
            nc.sync.dma_start(psb[:], params[:])
            b1_ap = psb[:, 0:1]
            lnw_ap = psb[:, 1:2]
            lnb_ap = psb[:, 2:3]
            b2_ap = psb[:, 3:4]
            eps_ap = psb[:, 4:5]

            process(0)
            for l in range(2, L + 1):
                if l < L:
                    load_slice(l)
                process(l - 1)

    nc.compile()
    return nc


def _get_program():
    key = MM_DTYPE
    if key not in _CACHE:
        _CACHE[key] = _build(key)
    return _CACHE[key]


def _host_prep(x, w1, b1, ln_w, ln_b, w2, b2):
    x = np.ascontiguousarray(np.asarray(x, dtype=np.float32))
    xm = x.reshape(N * C, T, L, H, W)
    # pad H and W by 1 on each side with zeros
    xpad = np.zeros((N * C, T, L, H + 2, W + 2), np.float32)
    xpad[:, :, :, 1:H + 1, 1:W + 1] = xm
    zslice = np.zeros((N * C, L, H + 2, W + 2), np.float32)
    xins = []
    for t in range(T):
        xp = xpad[:, t - 1] if t > 0 else zslice
        xc = xpad[:, t]
        xn = xpad[:, t + 1] if t < T - 1 else zslice
        xins.append(np.ascontiguousarray(np.stack([xp, xc, xn])))

    w1c = np.ascontiguousarray(
        np.asarray(w1, dtype=np.float32).transpose(1, 2, 3, 4, 5, 0)
    ).reshape(C, 81, C)
    w2t = np.asarray(w2, dtype=np.float32).reshape(C, C).T
    w2bd = np.zeros((P, P), np.float32)
    w2bd[:C, :C] = w2t
    w2bd[C:, C:] = w2t
    onesbc = np.zeros((P, P), np.float32)
    onesbc[:C, :C] = 1.0 / C
    onesbc[C:, C:] = 1.0 / C
    params = np.zeros((P, 5), np.float32)
    params[:, 0] = np.tile(np.asarray(b1, dtype=np.float32), 2)
    params[:, 1] = np.tile(np.asarray(ln_w, dtype=np.float32), 2)
    params[:, 2] = np.tile(np.asarray(ln_b, dtype=np.float32), 2)
    params[:, 3] = np.tile(np.asarray(b2, dtype=np.float32), 2)
    params[:, 4] = EPS
    return xins, w1c, w2bd, onesbc, params


def kernel(x, w1, b1, ln_w, ln_b, w2, b2):
    global LAST_RESULTS
    xins, w1c, w2bd, onesbc, params = _host_prep(
        x, w1, b1, ln_w, ln_b, w2, b2)
    if MM_DTYPE == "bfloat16":
        import ml_dtypes
        bf = ml_dtypes.bfloat16
        xins = [a.astype(bf) for a in xins]
        w1c, w2bd, onesbc = w1c.astype(bf), w2bd.astype(bf), onesbc.astype(bf)
    nc = _get_program()
    in_maps = [
        {"xinp": xins[t], "w1c": w1c, "w2bd": w2bd, "onesbc": onesbc,
         "params": params}
        for t in range(T)
    ]
    res = bass_utils.run_bass_kernel_spmd(
        nc, in_maps, core_ids=list(range(8)), trace=TRACE)
    LAST_RESULTS = res
    out = np.stack([res.results[t]["out"] for t in range(T)], axis=1)
    return np.ascontiguousarray(out.reshape(N, C, T, L, H, W))



# revision 8
# speedup vs baseline: 1.1622x; 1.0587x over previous
"""Trainium2 Bass kernel for a 4D ConvBlock (conv3^4 -> LN -> GELU -> 1x1 conv -> residual).

Strategy (8 NeuronCores, class-balanced work sharding):
  - Work unit = (t, H-half, L-range-of-4). 32 units globally; each core gets
    exactly 1 edge-t unit + 3 interior-t units = 1089 conv matmuls/core
    (perfectly balanced; a plain T-shard gives interior cores 1188 while
    edge cores idle 1/3 of their taps on zero halos).
  - Units with the upper L-range are processed L-flipped (host reorders
    slices and negates dl in the weight layout) so every unit uniformly has
    its L-edge at position 0. Same compiled program on all 8 cores; all
    per-core differences live in host-prepared input tensors.
  - Partition layout: 128 SBUF partitions = (sample n)*64 + channel c.
    conv1 = accumulating PE matmuls with BLOCK-DIAGONAL [128,128] weights so
    both batch samples ride one matmul.
  - Each input slice is stored as 3 pre-shifted dw-planes of [18,32] rows so
    every tap's matmul rhs is a single flat contiguous 512-element slice
    (contiguous moving operands stream ~6% faster than strided windows).
  - All matmul operands bf16 (PE full rate; fp8 DoubleRow measured NO
    faster per matmul wall-time, and plain fp8 fails the accuracy gate).
  - Channel-wise LayerNorm stats via block-diag ones matmuls (mean and
    mean-of-squares broadcast in one PE op each); exact-erf GELU on ACT.
  - Emission is software-pipelined A(g);A(g+1);B(g);A(g+2);B(g+1);C(g);...
    so the PE never stalls on the DVE/ACT LayerNorm chain between groups.
    PSUM budget: 2 acc + 4 stat + 2 out = exactly 8 banks.
"""
import os
import sys

os.environ.setdefault("MYCRO_LOCAL_CACHE", "1")
for _p in ("/opt/trn_rl_repo",):
    if os.path.isdir(_p) and _p not in sys.path:
        sys.path.insert(0, _p)

import numpy as np
import ml_dtypes

import concourse.bass as bass
import concourse.tile as tile
from concourse import bacc, mybir
from concourse import bass_utils

TRACE = os.environ.get("KERNEL_TRACE", "0") == "1"
MM_DTYPE = "bfloat16"  # test harness prints this

N, C, T, L, H, W = 2, 64, 8, 8, 32, 32
P = 128
EPS = 1e-5
NU = 4          # units per core
NG = 16         # acc groups per core (4 l-positions per unit)
NROW = 18       # input rows per half-slice (16 + dh halo)
FLAT = NROW * W # 576
NW_EDGE = 54    # edge-unit weight rows
NW_INT = 81     # interior-unit weight rows (shared by units 1..3)
NW = NW_EDGE + NW_INT  # 135

_CACHE = {}
LAST_RESULTS = None

# ---------------------------------------------------------------------------
# Unit table (global, fixed): unit = (t, half, lr). Edge units (t in {0,7})
# go one per core; interior units three per core.
_EDGE_UNITS = [(t, h, r) for t in (0, 7) for h in (0, 1) for r in (0, 1)]
# interior units ordered so each core's triple shares one lr (the three
# interior units of a core share one weight-row set, whose dl layout
# depends on the unit's l-flip)
_INT_UNITS = ([(t, h, 0) for t in range(1, 7) for h in (0, 1)] +
              [(t, h, 1) for t in range(1, 7) for h in (0, 1)])
UNITS = [[_EDGE_UNITS[k]] + _INT_UNITS[3 * k: 3 * k + 3] for k in range(8)]


def _unit_geometry(t, half, lr):
    """Returns (lvals, dt_list) for a unit. lvals[0] is the L-edge.
    dt_list[0] is always dt=0 (center)."""
    if lr == 0:
        lvals = [0, 1, 2, 3, 4]        # positions 0..3 + halo at index 4
        flip = 1
    else:
        lvals = [7, 6, 5, 4, 3]
        flip = -1
    if t == 0:
        dts = [0, 1]
    elif t == T - 1:
        dts = [0, -1]
    else:
        dts = [0, -1, 1]
    return lvals, dts, flip


def _build():
    f32 = mybir.dt.float32
    bf16 = mybir.dt.bfloat16
    AF = mybir.ActivationFunctionType

    nc = bacc.Bacc("TRN2", target_bir_lowering=False, debug=False,
                   enable_asserts=False, num_devices=8)
    # xin[u, d, s] = one input slice: 3 dw-planes x 18 rows x 32 cols
    xin = nc.dram_tensor("xin", [NU, 3, 5, P, 3, FLAT], bf16,
                         kind="ExternalInput").ap()
    w1c = nc.dram_tensor("w1c", [P, NW, P], bf16, kind="ExternalInput").ap()
    w2bd = nc.dram_tensor("w2bd", [P, P], bf16, kind="ExternalInput").ap()
    onesbc = nc.dram_tensor("onesbc", [P, P], bf16, kind="ExternalInput").ap()
    params = nc.dram_tensor("params", [P, 5], f32, kind="ExternalInput").ap()
    out = nc.dram_tensor("out", [P, NG, 512], f32, kind="ExternalOutput").ap()

    # weight row layout (must match host):
    #   edge rows:     (dls<2 first) idx = dls*18 + d*9 + dh*3 + dw   d in 0..1
    #   interior rows: 54 + dls*27 + d*9 + dh*3 + dw                  d in 0..2
    def wrow(u, dls, d, dh, dw):
        if u == 0:
            return dls * 18 + d * 9 + dh * 3 + dw
        return NW_EDGE + dls * 27 + d * 9 + dh * 3 + dw

    with tile.TileContext(nc) as tc:
        with (
            tc.tile_pool(name="wpool", bufs=1) as wpool,
            tc.tile_pool(name="xpool", bufs=2) as xpool,
            tc.tile_pool(name="work", bufs=2) as work,
            tc.tile_pool(name="respool", bufs=4) as respool,
            tc.tile_pool(name="ps_acc", bufs=2, space=bass.MemorySpace.PSUM) as ps_acc,
            tc.tile_pool(name="ps_bc", bufs=2, space=bass.MemorySpace.PSUM) as ps_bc,
            tc.tile_pool(name="ps_out", bufs=2, space=bass.MemorySpace.PSUM) as ps_out,
        ):
            # --- weights, chunked in consumption order -------------------
            w1sb = wpool.tile([P, NW, P], bf16, name="w1sb", tag="w1sb")
            wchunks = [(0, 36), (36, 54), (54, 108), (108, 135)]
            wloaded = 0

            def emit_wchunk():
                nonlocal wloaded
                a, b = wchunks[wloaded]
                nc.sync.dma_start(w1sb[:, a:b, :], w1c[:, a:b, :])
                wloaded += 1

            xs = {}

            def load_unit(u, dts, first_two_only=False, rest=False):
                srange = range(5)
                for d in range(len(dts)):
                    for s in srange:
                        if first_two_only and s >= 2:
                            continue
                        if rest and s < 2:
                            continue
                        xt = xpool.tile([P, 3, FLAT], bf16,
                                        name=f"x{u}_{d}_{s}", tag=f"x{d}_{s}")
                        nc.sync.dma_start(xt[:], xin[u, d, s])
                        xs[(u, d, s)] = xt

            psb = wpool.tile([P, 5], f32, name="psb", tag="psb")
            w2sb = wpool.tile([P, P], bf16, name="w2sb", tag="w2sb")
            onsb = wpool.tile([P, P], bf16, name="onsb", tag="onsb")

            def emit_smalls():
                nc.sync.dma_start(psb[:], params[:])
                nc.sync.dma_start(w2sb[:], w2bd[:])
                nc.sync.dma_start(onsb[:], onesbc[:])

            b1_ap = psb[:, 0:1]
            lnw_ap = psb[:, 1:2]
            lnb_ap = psb[:, 2:3]
            b2_ap = psb[:, 3:4]
            eps_ap = psb[:, 4:5]

            state = {}

            def stage_A(gi):
                u, g = divmod(gi, 4)
                ndt = 2 if u == 0 else 3
                acc = ps_acc.tile([P, 512], f32, name=f"acc{gi}", tag="acc")
                # dl-slots: 0 -> slice g, 1 -> slice g+1, 2 -> slice g-1
                dls_list = [0, 1] if g == 0 else [0, 1, 2]
                slices = {0: g, 1: g + 1, 2: g - 1}
                n = sum(1 for _ in dls_list) * ndt * 9
                i = 0
                for dls in dls_list:
                    for d in range(ndt):
                        xt = xs[(u, d, slices[dls])]
                        for dh in range(3):
                            base = dh * W
                            for dw in range(3):
                                nc.tensor.matmul(
                                    acc[:],
                                    w1sb[:, wrow(u, dls, d, dh, dw), :],
                                    xt[:, dw, base: base + 512],
                                    start=(i == 0), stop=(i == n - 1))
                                i += 1
                state[("acc", gi)] = acc

            def stage_D1B(gi):
                acc = state.pop(("acc", gi))
                h = work.tile([P, 512], bf16, name=f"h{gi}", tag="h")
                nc.vector.tensor_scalar_add(h[:], acc[:], b1_ap)
                sq = work.tile([P, 512], bf16, name=f"sq{gi}", tag="sq")
                nc.vector.tensor_mul(sq[:], h[:], h[:])
                bc_mu = ps_bc.tile([P, 512], f32, name=f"bcmu{gi}", tag="bc_mu")
                nc.tensor.matmul(bc_mu[:], onsb[:], h[:])
                bc_e2 = ps_bc.tile([P, 512], f32, name=f"bce2{gi}", tag="bc_e2")
                nc.tensor.matmul(bc_e2[:], onsb[:], sq[:])
                u, g = divmod(gi, 4)
                res = respool.tile([P, 512], bf16, name=f"res{gi}", tag="res")
                nc.vector.tensor_copy(res[:], xs[(u, 0, g)][:, 1, W: W + 512])
                state[("h", gi)] = h
                state[("bc", gi)] = (bc_mu, bc_e2)
                state[("res", gi)] = res

            def stage_D2C(gi):
                h = state.pop(("h", gi))
                bc_mu, bc_e2 = state.pop(("bc", gi))
                mu_sbf = work.tile([P, 512], f32, name=f"mu{gi}", tag="mu_sbf")
                nc.vector.tensor_copy(mu_sbf[:], bc_mu[:])
                mu2 = work.tile([P, 512], f32, name=f"mu2{gi}", tag="mu2")
                nc.vector.tensor_mul(mu2[:], mu_sbf[:], mu_sbf[:])
                var = work.tile([P, 512], f32, name=f"var{gi}", tag="var")
                nc.vector.tensor_sub(var[:], bc_e2[:], mu2[:])
                rstd = work.tile([P, 512], f32, name=f"rstd{gi}", tag="rstd")
                nc.scalar.activation(rstd[:], var[:], AF.Abs_reciprocal_sqrt,
                                     bias=eps_ap, scale=1.0)
                t1 = work.tile([P, 512], f32, name=f"t1{gi}", tag="t1")
                nc.vector.tensor_sub(t1[:], h[:], mu_sbf[:])
                t2 = work.tile([P, 512], f32, name=f"t2{gi}", tag="t2")
                nc.vector.tensor_mul(t2[:], t1[:], rstd[:])
                g8 = work.tile([P, 512], bf16, name=f"g8{gi}", tag="g8")
                nc.scalar.activation(g8[:], t2[:], AF.Gelu,
                                     bias=lnb_ap, scale=lnw_ap)
                ps2 = ps_out.tile([P, 512], f32, name=f"ps2{gi}", tag="ps2")
                nc.tensor.matmul(ps2[:], w2sb[:], g8[:])
                state[("ps2", gi)] = ps2

            def stage_D3(gi):
                u, g = divmod(gi, 4)
                ps2 = state.pop(("ps2", gi))
                o1 = work.tile([P, 512], f32, name=f"o1{gi}", tag="o1")
                nc.vector.tensor_scalar_add(o1[:], ps2[:], b2_ap)
                osb = work.tile([P, 512], f32, name=f"osb{gi}", tag="osb")
                res = state.pop(("res", gi))
                nc.vector.tensor_add(osb[:], o1[:], res[:])
                nc.sync.dma_start(out[:, gi, :], osb[:])

            # ----- emission schedule -------------------------------------
            # loads: unit0 g0-critical slices, weights chunk0, smalls, rest
            u0 = UNITS[0][0]  # geometry identical across cores
            _, dts0, _ = _unit_geometry(*u0)
            load_unit(0, dts0, first_two_only=True)
            emit_wchunk()            # rows 0..36 (edge g0 taps)
            emit_smalls()
            load_unit(0, dts0, rest=True)
            emit_wchunk()            # rows 36..54
            _, dtsI, _ = _unit_geometry(*UNITS[0][1])
            load_unit(1, dtsI, first_two_only=True)
            emit_wchunk()            # rows 54..108
            load_unit(1, dtsI, rest=True)
            emit_wchunk()            # rows 108..135

            for gi in range(NG):
                stage_A(gi)
                if gi >= 1:
                    stage_D1B(gi - 1)
                if gi >= 2:
                    stage_D2C(gi - 2)
                if gi >= 3:
                    stage_D3(gi - 3)
                if gi in (4, 8):
                    load_unit(gi // 4 + 1, dtsI)  # prefetch next unit
            stage_D1B(NG - 1)
            stage_D2C(NG - 2)
            stage_D3(NG - 3)
            stage_D2C(NG - 1)
            stage_D3(NG - 2)
            stage_D3(NG - 1)

    nc.compile()
    return nc


def _get_program():
    if "nc" not in _CACHE:
        _CACHE["nc"] = _build()
    return _CACHE["nc"]


def _host_prep(x, w1, b1, ln_w, ln_b, w2, b2):
    bf = ml_dtypes.bfloat16
    x = np.ascontiguousarray(np.asarray(x, dtype=np.float32))
    w1 = np.asarray(w1, dtype=np.float32)
    xm = x.reshape(N * C, T, L, H, W)
    # pad W by 1 on each side only (dw-planes); pad H rows on the fly
    xpadw = np.zeros((N * C, T, L, H, W + 2), np.float32)
    xpadw[:, :, :, :, 1:W + 1] = xm

    def slice_block(t, l, half):
        """[P, 3, NROW, W] bf16: 3 dw-planes of input rows for one half."""
        blk = np.zeros((P, 3, NROW, W), np.float32)
        if 0 <= t < T and 0 <= l < L:
            r0 = half * 16 - 1
            rows = xpadw[:, t, l]          # [P, H, W+2]
            lo, hi = max(r0, 0), min(r0 + NROW, H)
            for j in range(3):             # dw = j-1 -> col offset j
                blk[:, j, lo - r0: hi - r0, :] = rows[:, lo:hi, j: j + W]
        return blk.astype(bf)

    # per-core xin + w1c
    w1t = w1.transpose(1, 2, 3, 4, 5, 0)   # [Cin, dt, dl, dh, dw, Cout]
    xins, w1cs = [], []
    for k in range(8):
        xin = np.zeros((NU, 3, 5, P, 3, NROW, W), bf, order="C")
        rows_e = np.zeros((NW_EDGE, C, C), np.float32)
        rows_i = np.zeros((NW_INT, C, C), np.float32)
        for u, (t, half, lr) in enumerate(UNITS[k]):
            lvals, dts, flip = _unit_geometry(t, half, lr)
            for d, dt in enumerate(dts):
                for s in range(5):
                    xin[u, d, s] = slice_block(t + dt, lvals[s], half)
            # weight rows for this unit's class (edge u==0, interior shared)
            tgt = rows_e if u == 0 else rows_i
            for dls, dl in enumerate((0, 1, -1)):
                for d, dt in enumerate(dts):
                    for dh in range(3):
                        for dw in range(3):
                            r = dls * 9 * len(dts) + d * 9 + dh * 3 + dw
                            tgt[r] = w1t[:, dt + 1, flip * dl + 1,
                                         dh, dw, :]
        wfull = np.concatenate([rows_e, rows_i], axis=0)  # [NW, C, C]
        w1bd = np.zeros((NW, P, P), np.float32)
        w1bd[:, :C, :C] = wfull
        w1bd[:, C:, C:] = wfull
        xins.append(np.ascontiguousarray(xin.reshape(NU, 3, 5, P, 3, FLAT)))
        w1cs.append(np.ascontiguousarray(
            w1bd.transpose(1, 0, 2)).astype(bf))

    w2t = np.asarray(w2, dtype=np.float32).reshape(C, C).T
    w2bd = np.zeros((P, P), np.float32)
    w2bd[:C, :C] = w2t
    w2bd[C:, C:] = w2t
    onesbc = np.zeros((P, P), np.float32)
    onesbc[:C, :C] = 1.0 / C
    onesbc[C:, C:] = 1.0 / C
    params = np.zeros((P, 5), np.float32)
    params[:, 0] = np.tile(np.asarray(b1, dtype=np.float32), 2)
    params[:, 1] = np.tile(np.asarray(ln_w, dtype=np.float32), 2)
    params[:, 2] = np.tile(np.asarray(ln_b, dtype=np.float32), 2)
    params[:, 3] = np.tile(np.asarray(b2, dtype=np.float32), 2)
    params[:, 4] = EPS
    return xins, w1cs, w2bd.astype(bf), onesbc.astype(bf), params


def kernel(x, w1, b1, ln_w, ln_b, w2, b2):
    global LAST_RESULTS
    xins, w1cs, w2bd, onesbc, params = _host_prep(
        x, w1, b1, ln_w, ln_b, w2, b2)
    nc = _get_program()
    in_maps = [
        {"xin": xins[k], "w1c": w1cs[k], "w2bd": w2bd, "onesbc": onesbc,
         "params": params}
        for k in range(8)
    ]
    res = bass_utils.run_bass_kernel_spmd(
        nc, in_maps, core_ids=list(range(8)), trace=TRACE)
    LAST_RESULTS = res
    out = np.zeros((N, C, T, L, H, W), np.float32)
    for k in range(8):
        o = res.results[k]["out"]          # [P, NG, 512]
        for u, (t, half, lr) in enumerate(UNITS[k]):
            lvals, _, _ = _unit_geometry(t, half, lr)
            for g in range(4):
                blk = o[:, u * 4 + g].reshape(N, C, 16, W)
                out[:, :, t, lvals[g], 16 * half:16 * half + 16, :] = blk
    return np.ascontiguousarray(out)


# revision 9
# speedup vs baseline: 1.1902x; 1.0241x over previous
"""Trainium2 Bass kernel for a 4D ConvBlock (conv3^4 -> LN -> GELU -> 1x1 conv -> residual).

Strategy (8 NeuronCores, class-balanced work sharding):
  - Work unit = (t, H-half, L-range-of-4). 32 units globally; each core gets
    exactly 1 edge-t unit + 3 interior-t units = 1089 conv matmuls/core
    (perfectly balanced; a plain T-shard gives interior cores 1188 while
    edge cores idle 1/3 of their taps on zero halos).
  - Units with the upper L-range are processed L-flipped (host reorders
    slices and negates dl in the weight layout) so every unit uniformly has
    its L-edge at position 0. Same compiled program on all 8 cores; all
    per-core differences live in host-prepared input tensors.
  - Partition layout: 128 SBUF partitions = (sample n)*64 + channel c.
    conv1 = accumulating PE matmuls with BLOCK-DIAGONAL [128,128] weights so
    both batch samples ride one matmul.
  - Each input slice is stored as 3 pre-shifted dw-planes of [18,32] rows so
    every tap's matmul rhs is a single flat contiguous 512-element slice
    (contiguous moving operands stream ~6% faster than strided windows).
  - All matmul operands bf16 (PE full rate; fp8 DoubleRow measured NO
    faster per matmul wall-time, and plain fp8 fails the accuracy gate).
  - Channel-wise LayerNorm stats via block-diag ones matmuls (mean and
    mean-of-squares broadcast in one PE op each); exact-erf GELU on ACT.
  - Emission is software-pipelined A(g);A(g+1);B(g);A(g+2);B(g+1);C(g);...
    so the PE never stalls on the DVE/ACT LayerNorm chain between groups.
    PSUM budget: 2 acc + 4 stat + 2 out = exactly 8 banks.
"""
import os
import sys

os.environ.setdefault("MYCRO_LOCAL_CACHE", "1")
for _p in ("/opt/trn_rl_repo",):
    if os.path.isdir(_p) and _p not in sys.path:
        sys.path.insert(0, _p)

import numpy as np
import ml_dtypes

import concourse.bass as bass
import concourse.tile as tile
from concourse import bacc, mybir
from concourse import bass_utils

TRACE = os.environ.get("KERNEL_TRACE", "0") == "1"
MM_DTYPE = "bfloat16"  # test harness prints this

N, C, T, L, H, W = 2, 64, 8, 8, 32, 32
P = 128
EPS = 1e-5
NU = 4          # units per core
NG = 16         # acc groups per core (4 l-positions per unit)
NROW = 18       # input rows per half-slice (16 + dh halo)
FLAT = NROW * W # 576
NW_EDGE = 54    # edge-unit weight rows
NW_INT = 81     # interior-unit weight rows (shared by units 1..3)
NW = NW_EDGE + NW_INT  # 135

_CACHE = {}
LAST_RESULTS = None

# ---------------------------------------------------------------------------
# Unit table (global, fixed): unit = (t, half, lr). Edge units (t in {0,7})
# go one per core; interior units three per core.
_EDGE_UNITS = [(t, h, r) for t in (0, 7) for h in (0, 1) for r in (0, 1)]
# interior units ordered so each core's triple shares one lr (the three
# interior units of a core share one weight-row set, whose dl layout
# depends on the unit's l-flip)
_INT_UNITS = ([(t, h, 0) for t in range(1, 7) for h in (0, 1)] +
              [(t, h, 1) for t in range(1, 7) for h in (0, 1)])
UNITS = [[_EDGE_UNITS[k]] + _INT_UNITS[3 * k: 3 * k + 3] for k in range(8)]


def _unit_geometry(t, half, lr):
    """Returns (lvals, dt_list) for a unit. lvals[0] is the L-edge.
    dt_list[0] is always dt=0 (center)."""
    if lr == 0:
        lvals = [0, 1, 2, 3, 4]        # positions 0..3 + halo at index 4
        flip = 1
    else:
        lvals = [7, 6, 5, 4, 3]
        flip = -1
    if t == 0:
        dts = [0, 1]
    elif t == T - 1:
        dts = [0, -1]
    else:
        dts = [0, -1, 1]
    return lvals, dts, flip


def _build():
    f32 = mybir.dt.float32
    bf16 = mybir.dt.bfloat16
    AF = mybir.ActivationFunctionType

    nc = bacc.Bacc("TRN2", target_bir_lowering=False, debug=False,
                   enable_asserts=False, num_devices=8)
    # xin[u, d, s] = one input slice: 3 dw-planes x 18 rows x 32 cols
    xin = nc.dram_tensor("xin", [NU, 3, 5, P, 3, FLAT], bf16,
                         kind="ExternalInput").ap()
    w1c = nc.dram_tensor("w1c", [P, NW, P], bf16, kind="ExternalInput").ap()
    w2bd = nc.dram_tensor("w2bd", [P, P], bf16, kind="ExternalInput").ap()
    onesbc = nc.dram_tensor("onesbc", [P, P], bf16, kind="ExternalInput").ap()
    params = nc.dram_tensor("params", [P, 5], f32, kind="ExternalInput").ap()
    out = nc.dram_tensor("out", [P, NG, 512], f32, kind="ExternalOutput").ap()

    # weight row layout (must match host):
    #   edge rows:     (dls<2 first) idx = dls*18 + d*9 + dh*3 + dw   d in 0..1
    #   interior rows: 54 + dls*27 + d*9 + dh*3 + dw                  d in 0..2
    def wrow(u, dls, d, dh, dw):
        if u == 0:
            return dls * 18 + d * 9 + dh * 3 + dw
        return NW_EDGE + dls * 27 + d * 9 + dh * 3 + dw

    with tile.TileContext(nc) as tc:
        with (
            tc.tile_pool(name="wpool", bufs=1) as wpool,
            tc.tile_pool(name="xpool", bufs=2) as xpool,
            tc.tile_pool(name="work", bufs=2) as work,
            tc.tile_pool(name="respool", bufs=4) as respool,
            tc.tile_pool(name="ps_acc", bufs=2, space=bass.MemorySpace.PSUM) as ps_acc,
            tc.tile_pool(name="ps_bc", bufs=2, space=bass.MemorySpace.PSUM) as ps_bc,
            tc.tile_pool(name="ps_out", bufs=2, space=bass.MemorySpace.PSUM) as ps_out,
        ):
            # --- weights, chunked in consumption order -------------------
            w1sb = wpool.tile([P, NW, P], bf16, name="w1sb", tag="w1sb")
            wchunks = [(0, 18), (18, 36), (36, 54), (54, 108), (108, 135)]
            wloaded = 0

            def emit_wchunk():
                nonlocal wloaded
                a, b = wchunks[wloaded]
                nc.sync.dma_start(w1sb[:, a:b, :], w1c[:, a:b, :])
                wloaded += 1

            xs = {}

            def load_unit(u, dts, only_s=None):
                for s in (range(5) if only_s is None else only_s):
                    for d in range(len(dts)):
                        xt = xpool.tile([P, 3, FLAT], bf16,
                                        name=f"x{u}_{d}_{s}", tag=f"x{d}_{s}")
                        nc.sync.dma_start(xt[:], xin[u, d, s])
                        xs[(u, d, s)] = xt

            psb = wpool.tile([P, 5], f32, name="psb", tag="psb")
            w2sb = wpool.tile([P, P], bf16, name="w2sb", tag="w2sb")
            onsb = wpool.tile([P, P], bf16, name="onsb", tag="onsb")

            def emit_smalls():
                nc.sync.dma_start(psb[:], params[:])
                nc.sync.dma_start(w2sb[:], w2bd[:])
                nc.sync.dma_start(onsb[:], onesbc[:])

            b1_ap = psb[:, 0:1]
            lnw_ap = psb[:, 1:2]
            lnb_ap = psb[:, 2:3]
            b2_ap = psb[:, 3:4]
            eps_ap = psb[:, 4:5]

            state = {}

            def stage_A(gi):
                u, g = divmod(gi, 4)
                ndt = 2 if u == 0 else 3
                acc = ps_acc.tile([P, 512], f32, name=f"acc{gi}", tag="acc")
                # dl-slots: 0 -> slice g, 1 -> slice g+1, 2 -> slice g-1
                dls_list = [0, 1] if g == 0 else [0, 1, 2]
                slices = {0: g, 1: g + 1, 2: g - 1}
                n = sum(1 for _ in dls_list) * ndt * 9
                i = 0
                for dls in dls_list:
                    for d in range(ndt):
                        xt = xs[(u, d, slices[dls])]
                        for dh in range(3):
                            base = dh * W
                            for dw in range(3):
                                nc.tensor.matmul(
                                    acc[:],
                                    w1sb[:, wrow(u, dls, d, dh, dw), :],
                                    xt[:, dw, base: base + 512],
                                    start=(i == 0), stop=(i == n - 1))
                                i += 1
                state[("acc", gi)] = acc

            def stage_D1B(gi):
                acc = state.pop(("acc", gi))
                h = work.tile([P, 512], bf16, name=f"h{gi}", tag="h")
                nc.vector.tensor_scalar_add(h[:], acc[:], b1_ap)
                sq = work.tile([P, 512], bf16, name=f"sq{gi}", tag="sq")
                nc.vector.tensor_mul(sq[:], h[:], h[:])
                bc_mu = ps_bc.tile([P, 512], f32, name=f"bcmu{gi}", tag="bc_mu")
                nc.tensor.matmul(bc_mu[:], onsb[:], h[:])
                bc_e2 = ps_bc.tile([P, 512], f32, name=f"bce2{gi}", tag="bc_e2")
                nc.tensor.matmul(bc_e2[:], onsb[:], sq[:])
                u, g = divmod(gi, 4)
                res = respool.tile([P, 512], bf16, name=f"res{gi}", tag="res")
                nc.vector.tensor_copy(res[:], xs[(u, 0, g)][:, 1, W: W + 512])
                state[("h", gi)] = h
                state[("bc", gi)] = (bc_mu, bc_e2)
                state[("res", gi)] = res

            def stage_D2C_pair(ga, gb):
                # var chains for both groups, then rstd+rstd / gelu+gelu so
                # the ACT engine swaps its function table 2x per pair, not 4x
                hs, mus, vars_, rstds = {}, {}, {}, {}
                for gi in (ga, gb):
                    h = state.pop(("h", gi))
                    bc_mu, bc_e2 = state.pop(("bc", gi))
                    mu_sbf = work.tile([P, 512], f32, name=f"mu{gi}",
                                       tag="mu_sbf")
                    nc.vector.tensor_copy(mu_sbf[:], bc_mu[:])
                    mu2 = work.tile([P, 512], f32, name=f"mu2{gi}", tag="mu2")
                    nc.vector.tensor_mul(mu2[:], mu_sbf[:], mu_sbf[:])
                    var = work.tile([P, 512], f32, name=f"var{gi}", tag="var")
                    nc.vector.tensor_sub(var[:], bc_e2[:], mu2[:])
                    hs[gi], mus[gi], vars_[gi] = h, mu_sbf, var
                for gi in (ga, gb):
                    rstd = work.tile([P, 512], f32, name=f"rstd{gi}",
                                     tag="rstd")
                    nc.scalar.activation(rstd[:], vars_[gi][:],
                                         AF.Abs_reciprocal_sqrt,
                                         bias=eps_ap, scale=1.0)
                    rstds[gi] = rstd
                t2s = {}
                for gi in (ga, gb):
                    t1 = work.tile([P, 512], f32, name=f"t1{gi}", tag="t1")
                    nc.vector.tensor_sub(t1[:], hs[gi][:], mus[gi][:])
                    t2 = work.tile([P, 512], f32, name=f"t2{gi}", tag="t2")
                    nc.vector.tensor_mul(t2[:], t1[:], rstds[gi][:])
                    t2s[gi] = t2
                for gi in (ga, gb):
                    g8 = work.tile([P, 512], bf16, name=f"g8{gi}", tag="g8")
                    nc.scalar.activation(g8[:], t2s[gi][:], AF.Gelu,
                                         bias=lnb_ap, scale=lnw_ap)
                    ps2 = ps_out.tile([P, 512], f32, name=f"ps2{gi}",
                                      tag="ps2")
                    nc.tensor.matmul(ps2[:], w2sb[:], g8[:])
                    state[("ps2", gi)] = ps2

            def stage_D3(gi):
                u, g = divmod(gi, 4)
                ps2 = state.pop(("ps2", gi))
                o1 = work.tile([P, 512], f32, name=f"o1{gi}", tag="o1")
                nc.vector.tensor_scalar_add(o1[:], ps2[:], b2_ap)
                osb = work.tile([P, 512], f32, name=f"osb{gi}", tag="osb")
                res = state.pop(("res", gi))
                nc.vector.tensor_add(osb[:], o1[:], res[:])
                nc.sync.dma_start(out[:, gi, :], osb[:])

            # ----- emission schedule -------------------------------------
            # loads: unit0 g0-critical slices, weights chunk0, smalls, rest
            u0 = UNITS[0][0]  # geometry identical across cores
            _, dts0, _ = _unit_geometry(*u0)
            # critical path for mm #0: slice (d,0) + weight rows 0..18
            load_unit(0, dts0, only_s=(0,))
            emit_wchunk()            # rows 0..18   (g0, dls=0 taps)
            load_unit(0, dts0, only_s=(1,))
            emit_wchunk()            # rows 18..36  (g0, dls=1 taps)
            emit_smalls()
            load_unit(0, dts0, only_s=(2, 3, 4))
            emit_wchunk()            # rows 36..54
            _, dtsI, _ = _unit_geometry(*UNITS[0][1])
            load_unit(1, dtsI, only_s=(0, 1))
            emit_wchunk()            # rows 54..108
            load_unit(1, dtsI, only_s=(2, 3, 4))
            emit_wchunk()            # rows 108..135

            for gi in range(NG):
                stage_A(gi)
                if gi >= 1:
                    stage_D1B(gi - 1)
                if gi >= 2 and gi % 2 == 0:
                    stage_D2C_pair(gi - 2, gi - 1)
                    stage_D3(gi - 2)
                    stage_D3(gi - 1)
                if gi in (4, 8):
                    load_unit(gi // 4 + 1, dtsI)  # prefetch next unit
            stage_D1B(NG - 1)
            stage_D2C_pair(NG - 2, NG - 1)
            stage_D3(NG - 2)
            stage_D3(NG - 1)

    nc.compile()
    return nc


def _get_program():
    if "nc" not in _CACHE:
        _CACHE["nc"] = _build()
    return _CACHE["nc"]


def _host_prep(x, w1, b1, ln_w, ln_b, w2, b2):
    bf = ml_dtypes.bfloat16
    x = np.ascontiguousarray(np.asarray(x, dtype=np.float32))
    w1 = np.asarray(w1, dtype=np.float32)
    xm = x.reshape(N * C, T, L, H, W)
    # pad W by 1 on each side only (dw-planes); pad H rows on the fly
    xpadw = np.zeros((N * C, T, L, H, W + 2), np.float32)
    xpadw[:, :, :, :, 1:W + 1] = xm

    def slice_block(t, l, half):
        """[P, 3, NROW, W] bf16: 3 dw-planes of input rows for one half."""
        blk = np.zeros((P, 3, NROW, W), np.float32)
        if 0 <= t < T and 0 <= l < L:
            r0 = half * 16 - 1
            rows = xpadw[:, t, l]          # [P, H, W+2]
            lo, hi = max(r0, 0), min(r0 + NROW, H)
            for j in range(3):             # dw = j-1 -> col offset j
                blk[:, j, lo - r0: hi - r0, :] = rows[:, lo:hi, j: j + W]
        return blk.astype(bf)

    # per-core xin + w1c
    w1t = w1.transpose(1, 2, 3, 4, 5, 0)   # [Cin, dt, dl, dh, dw, Cout]
    xins, w1cs = [], []
    for k in range(8):
        xin = np.zeros((NU, 3, 5, P, 3, NROW, W), bf, order="C")
        rows_e = np.zeros((NW_EDGE, C, C), np.float32)
        rows_i = np.zeros((NW_INT, C, C), np.float32)
        for u, (t, half, lr) in enumerate(UNITS[k]):
            lvals, dts, flip = _unit_geometry(t, half, lr)
            for d, dt in enumerate(dts):
                for s in range(5):
                    xin[u, d, s] = slice_block(t + dt, lvals[s], half)
            # weight rows for this unit's class (edge u==0, interior shared)
            tgt = rows_e if u == 0 else rows_i
            for dls, dl in enumerate((0, 1, -1)):
                for d, dt in enumerate(dts):
                    for dh in range(3):
                        for dw in range(3):
                            r = dls * 9 * len(dts) + d * 9 + dh * 3 + dw
                            tgt[r] = w1t[:, dt + 1, flip * dl + 1,
                                         dh, dw, :]
        wfull = np.concatenate([rows_e, rows_i], axis=0)  # [NW, C, C]
        w1bd = np.zeros((NW, P, P), np.float32)
        w1bd[:, :C, :C] = wfull
        w1bd[:, C:, C:] = wfull
        xins.append(np.ascontiguousarray(xin.reshape(NU, 3, 5, P, 3, FLAT)))
        w1cs.append(np.ascontiguousarray(
            w1bd.transpose(1, 0, 2)).astype(bf))

    w2t = np.asarray(w2, dtype=np.float32).reshape(C, C).T
    w2bd = np.zeros((P, P), np.float32)
    w2bd[:C, :C] = w2t
    w2bd[C:, C:] = w2t
    onesbc = np.zeros((P, P), np.float32)
    onesbc[:C, :C] = 1.0 / C
    onesbc[C:, C:] = 1.0 / C
    params = np.zeros((P, 5), np.float32)
    params[:, 0] = np.tile(np.asarray(b1, dtype=np.float32), 2)
    params[:, 1] = np.tile(np.asarray(ln_w, dtype=np.float32), 2)
    params[:, 2] = np.tile(np.asarray(ln_b, dtype=np.float32), 2)
    params[:, 3] = np.tile(np.asarray(b2, dtype=np.float32), 2)
    params[:, 4] = EPS
    return xins, w1cs, w2bd.astype(bf), onesbc.astype(bf), params


def kernel(x, w1, b1, ln_w, ln_b, w2, b2):
    global LAST_RESULTS
    xins, w1cs, w2bd, onesbc, params = _host_prep(
        x, w1, b1, ln_w, ln_b, w2, b2)
    nc = _get_program()
    in_maps = [
        {"xin": xins[k], "w1c": w1cs[k], "w2bd": w2bd, "onesbc": onesbc,
         "params": params}
        for k in range(8)
    ]
    res = bass_utils.run_bass_kernel_spmd(
        nc, in_maps, core_ids=list(range(8)), trace=TRACE)
    LAST_RESULTS = res
    out = np.zeros((N, C, T, L, H, W), np.float32)
    for k in range(8):
        o = res.results[k]["out"]          # [P, NG, 512]
        for u, (t, half, lr) in enumerate(UNITS[k]):
            lvals, _, _ = _unit_geometry(t, half, lr)
            for g in range(4):
                blk = o[:, u * 4 + g].reshape(N, C, 16, W)
                out[:, :, t, lvals[g], 16 * half:16 * half + 16, :] = blk
    return np.ascontiguousarray(out)


# revision 10
# speedup vs baseline: 1.1939x; 1.0031x over previous
"""Trainium2 Bass kernel for a 4D ConvBlock (conv3^4 -> LN -> GELU -> 1x1 conv -> residual).

Strategy (8 NeuronCores, class-balanced work sharding):
  - Work unit = (t, H-half, L-range-of-4). 32 units globally; each core gets
    exactly 1 edge-t unit + 3 interior-t units = 1089 conv matmuls/core
    (perfectly balanced; a plain T-shard gives interior cores 1188 while
    edge cores idle 1/3 of their taps on zero halos).
  - Units with the upper L-range are processed L-flipped (host reorders
    slices and negates dl in the weight layout) so every unit uniformly has
    its L-edge at position 0. Same compiled program on all 8 cores; all
    per-core differences live in host-prepared input tensors.
  - Partition layout: 128 SBUF partitions = (sample n)*64 + channel c.
    conv1 = accumulating PE matmuls with BLOCK-DIAGONAL [128,128] weights so
    both batch samples ride one matmul.
  - Each input slice is stored as 3 pre-shifted dw-planes of [18,32] rows so
    every tap's matmul rhs is a single flat contiguous 512-element slice
    (contiguous moving operands stream ~6% faster than strided windows).
  - All matmul operands bf16 (PE full rate; fp8 DoubleRow measured NO
    faster per matmul wall-time, and plain fp8 fails the accuracy gate).
  - Channel-wise LayerNorm stats via block-diag ones matmuls (mean and
    mean-of-squares broadcast in one PE op each); exact-erf GELU on ACT.
  - Emission is software-pipelined A(g);A(g+1);B(g);A(g+2);B(g+1);C(g);...
    so the PE never stalls on the DVE/ACT LayerNorm chain between groups.
    PSUM budget: 2 acc + 4 stat + 2 out = exactly 8 banks.
"""
import os
import sys

os.environ.setdefault("MYCRO_LOCAL_CACHE", "1")
for _p in ("/opt/trn_rl_repo",):
    if os.path.isdir(_p) and _p not in sys.path:
        sys.path.insert(0, _p)

import numpy as np
import ml_dtypes

import concourse.bass as bass
import concourse.tile as tile
from concourse import bacc, mybir
from concourse import bass_utils

TRACE = os.environ.get("KERNEL_TRACE", "0") == "1"
MM_DTYPE = "bfloat16"  # test harness prints this

N, C, T, L, H, W = 2, 64, 8, 8, 32, 32
P = 128
EPS = 1e-5
NU = 4          # units per core
NG = 16         # acc groups per core (4 l-positions per unit)
NROW = 18       # input rows per half-slice (16 + dh halo)
FLAT = NROW * W # 576
NW_EDGE = 54    # edge-unit weight rows
NW_INT = 81     # interior-unit weight rows (shared by units 1..3)
NW = NW_EDGE + NW_INT  # 135

_CACHE = {}
LAST_RESULTS = None

# ---------------------------------------------------------------------------
# Unit table (global, fixed): unit = (t, half, lr). Edge units (t in {0,7})
# go one per core; interior units three per core.
_EDGE_UNITS = [(t, h, r) for t in (0, 7) for h in (0, 1) for r in (0, 1)]
# interior units ordered so each core's triple shares one lr (the three
# interior units of a core share one weight-row set, whose dl layout
# depends on the unit's l-flip)
_INT_UNITS = ([(t, h, 0) for t in range(1, 7) for h in (0, 1)] +
              [(t, h, 1) for t in range(1, 7) for h in (0, 1)])
UNITS = [[_EDGE_UNITS[k]] + _INT_UNITS[3 * k: 3 * k + 3] for k in range(8)]


def _unit_geometry(t, half, lr):
    """Returns (lvals, dt_list) for a unit. lvals[0] is the L-edge.
    dt_list[0] is always dt=0 (center)."""
    if lr == 0:
        lvals = [0, 1, 2, 3, 4]        # positions 0..3 + halo at index 4
        flip = 1
    else:
        lvals = [7, 6, 5, 4, 3]
        flip = -1
    if t == 0:
        dts = [0, 1]
    elif t == T - 1:
        dts = [0, -1]
    else:
        dts = [0, -1, 1]
    return lvals, dts, flip


def _build():
    f32 = mybir.dt.float32
    bf16 = mybir.dt.bfloat16
    AF = mybir.ActivationFunctionType

    nc = bacc.Bacc("TRN2", target_bir_lowering=False, debug=False,
                   enable_asserts=False, num_devices=8)
    # xin[u, d, s] = one input slice: 3 dw-planes x 18 rows x 32 cols
    xin = nc.dram_tensor("xin", [NU, 3, 5, P, 3, FLAT], bf16,
                         kind="ExternalInput").ap()
    w1c = nc.dram_tensor("w1c", [P, NW, P], bf16, kind="ExternalInput").ap()
    w2bd = nc.dram_tensor("w2bd", [P, P], bf16, kind="ExternalInput").ap()
    onesbc = nc.dram_tensor("onesbc", [P, P], bf16, kind="ExternalInput").ap()
    params = nc.dram_tensor("params", [P, 5], f32, kind="ExternalInput").ap()
    out = nc.dram_tensor("out", [P, NG, 512], f32, kind="ExternalOutput").ap()

    # weight row layout (must match host):
    #   edge rows:     (dls<2 first) idx = dls*18 + d*9 + dh*3 + dw   d in 0..1
    #   interior rows: 54 + dls*27 + d*9 + dh*3 + dw                  d in 0..2
    def wrow(u, dls, d, dh, dw):
        if u == 0:
            return dls * 18 + d * 9 + dh * 3 + dw
        return NW_EDGE + dls * 27 + d * 9 + dh * 3 + dw

    with tile.TileContext(nc) as tc:
        with (
            tc.tile_pool(name="wpool", bufs=1) as wpool,
            tc.tile_pool(name="xpool", bufs=2) as xpool,
            tc.tile_pool(name="work", bufs=2) as work,
            tc.tile_pool(name="respool", bufs=4) as respool,
            tc.tile_pool(name="ps_acc", bufs=2, space=bass.MemorySpace.PSUM) as ps_acc,
            tc.tile_pool(name="ps_bc", bufs=2, space=bass.MemorySpace.PSUM) as ps_bc,
            tc.tile_pool(name="ps_out", bufs=2, space=bass.MemorySpace.PSUM) as ps_out,
        ):
            # --- weights, chunked in consumption order -------------------
            w1sb = wpool.tile([P, NW, P], bf16, name="w1sb", tag="w1sb")
            wchunks = [(0, 18), (18, 36), (36, 54), (54, 108), (108, 135)]
            wloaded = 0

            def emit_wchunk():
                nonlocal wloaded
                a, b = wchunks[wloaded]
                nc.sync.dma_start(w1sb[:, a:b, :], w1c[:, a:b, :])
                wloaded += 1

            xs = {}

            def load_unit(u, dts, only_s=None):
                for s in (range(5) if only_s is None else only_s):
                    for d in range(len(dts)):
                        xt = xpool.tile([P, 3, FLAT], bf16,
                                        name=f"x{u}_{d}_{s}", tag=f"x{d}_{s}")
                        nc.sync.dma_start(xt[:], xin[u, d, s])
                        xs[(u, d, s)] = xt

            psb = wpool.tile([P, 5], f32, name="psb", tag="psb")
            w2sb = wpool.tile([P, P], bf16, name="w2sb", tag="w2sb")
            onsb = wpool.tile([P, P], bf16, name="onsb", tag="onsb")

            def emit_smalls():
                nc.sync.dma_start(psb[:], params[:])
                nc.sync.dma_start(w2sb[:], w2bd[:])
                nc.sync.dma_start(onsb[:], onesbc[:])

            b1_ap = psb[:, 0:1]
            lnw_ap = psb[:, 1:2]
            lnb_ap = psb[:, 2:3]
            b2_ap = psb[:, 3:4]
            eps_ap = psb[:, 4:5]

            state = {}

            def stage_A(gi):
                u, g = divmod(gi, 4)
                ndt = 2 if u == 0 else 3
                acc = ps_acc.tile([P, 512], f32, name=f"acc{gi}", tag="acc")
                # dl-slots: 0 -> slice g, 1 -> slice g+1, 2 -> slice g-1
                dls_list = [0, 1] if g == 0 else [0, 1, 2]
                slices = {0: g, 1: g + 1, 2: g - 1}
                n = sum(1 for _ in dls_list) * ndt * 9
                i = 0
                for dls in dls_list:
                    for d in range(ndt):
                        xt = xs[(u, d, slices[dls])]
                        for dh in range(3):
                            base = dh * W
                            for dw in range(3):
                                nc.tensor.matmul(
                                    acc[:],
                                    w1sb[:, wrow(u, dls, d, dh, dw), :],
                                    xt[:, dw, base: base + 512],
                                    start=(i == 0), stop=(i == n - 1))
                                i += 1
                state[("acc", gi)] = acc

            def stage_D1B(gi):
                acc = state.pop(("acc", gi))
                h = work.tile([P, 512], bf16, name=f"h{gi}", tag="h")
                nc.vector.tensor_scalar_add(h[:], acc[:], b1_ap)
                sq = work.tile([P, 512], bf16, name=f"sq{gi}", tag="sq")
                nc.vector.tensor_mul(sq[:], h[:], h[:])
                bc_mu = ps_bc.tile([P, 512], f32, name=f"bcmu{gi}", tag="bc_mu")
                nc.tensor.matmul(bc_mu[:], onsb[:], h[:])
                bc_e2 = ps_bc.tile([P, 512], f32, name=f"bce2{gi}", tag="bc_e2")
                nc.tensor.matmul(bc_e2[:], onsb[:], sq[:])
                u, g = divmod(gi, 4)
                res = respool.tile([P, 512], bf16, name=f"res{gi}", tag="res")
                nc.vector.tensor_copy(res[:], xs[(u, 0, g)][:, 1, W: W + 512])
                state[("h", gi)] = h
                state[("bc", gi)] = (bc_mu, bc_e2)
                state[("res", gi)] = res

            def stage_D2C_pair(ga, gb):
                # var chains for both groups, then rstd+rstd / gelu+gelu so
                # the ACT engine swaps its function table 2x per pair, not 4x
                hs, mus, vars_, rstds = {}, {}, {}, {}
                for gi in (ga, gb):
                    h = state.pop(("h", gi))
                    bc_mu, bc_e2 = state.pop(("bc", gi))
                    mu_sbf = work.tile([P, 512], f32, name=f"mu{gi}",
                                       tag="mu_sbf")
                    nc.vector.tensor_copy(mu_sbf[:], bc_mu[:])
                    mu2 = work.tile([P, 512], f32, name=f"mu2{gi}", tag="mu2")
                    nc.vector.tensor_mul(mu2[:], mu_sbf[:], mu_sbf[:])
                    var = work.tile([P, 512], f32, name=f"var{gi}", tag="var")
                    nc.vector.tensor_sub(var[:], bc_e2[:], mu2[:])
                    hs[gi], mus[gi], vars_[gi] = h, mu_sbf, var
                rstd_is = []
                for gi in (ga, gb):
                    rstd = work.tile([P, 512], f32, name=f"rstd{gi}",
                                     tag="rstd")
                    rstd_is.append(nc.scalar.activation(
                        rstd[:], vars_[gi][:], AF.Abs_reciprocal_sqrt,
                        bias=eps_ap, scale=1.0))
                    rstds[gi] = rstd
                t2s = {}
                for gi in (ga, gb):
                    t1 = work.tile([P, 512], f32, name=f"t1{gi}", tag="t1")
                    nc.vector.tensor_sub(t1[:], hs[gi][:], mus[gi][:])
                    t2 = work.tile([P, 512], f32, name=f"t2{gi}", tag="t2")
                    nc.vector.tensor_mul(t2[:], t1[:], rstds[gi][:])
                    t2s[gi] = t2
                first_gelu = True
                for gi in (ga, gb):
                    g8 = work.tile([P, 512], bf16, name=f"g8{gi}", tag="g8")
                    gelu_i = nc.scalar.activation(g8[:], t2s[gi][:], AF.Gelu,
                                                  bias=lnb_ap, scale=lnw_ap)
                    if first_gelu:
                        tile.add_dep_helper(
                            gelu_i.ins, rstd_is[1].ins, sync=True,
                            reason="batch ACT tables: absr,absr,gelu,gelu")
                        first_gelu = False
                    ps2 = ps_out.tile([P, 512], f32, name=f"ps2{gi}",
                                      tag="ps2")
                    nc.tensor.matmul(ps2[:], w2sb[:], g8[:])
                    state[("ps2", gi)] = ps2

            def stage_D3(gi):
                u, g = divmod(gi, 4)
                ps2 = state.pop(("ps2", gi))
                o1 = work.tile([P, 512], f32, name=f"o1{gi}", tag="o1")
                nc.vector.tensor_scalar_add(o1[:], ps2[:], b2_ap)
                osb = work.tile([P, 512], f32, name=f"osb{gi}", tag="osb")
                res = state.pop(("res", gi))
                nc.vector.tensor_add(osb[:], o1[:], res[:])
                nc.sync.dma_start(out[:, gi, :], osb[:])

            # ----- emission schedule -------------------------------------
            # loads: unit0 g0-critical slices, weights chunk0, smalls, rest
            u0 = UNITS[0][0]  # geometry identical across cores
            _, dts0, _ = _unit_geometry(*u0)
            # critical path for mm #0: slice (d,0) + weight rows 0..18
            load_unit(0, dts0, only_s=(0,))
            emit_wchunk()            # rows 0..18   (g0, dls=0 taps)
            load_unit(0, dts0, only_s=(1,))
            emit_wchunk()            # rows 18..36  (g0, dls=1 taps)
            emit_smalls()
            load_unit(0, dts0, only_s=(2, 3, 4))
            emit_wchunk()            # rows 36..54
            _, dtsI, _ = _unit_geometry(*UNITS[0][1])
            load_unit(1, dtsI, only_s=(0, 1))
            emit_wchunk()            # rows 54..108
            load_unit(1, dtsI, only_s=(2, 3, 4))
            emit_wchunk()            # rows 108..135

            for gi in range(NG):
                stage_A(gi)
                if gi >= 1:
                    stage_D1B(gi - 1)
                if gi >= 2 and gi % 2 == 0:
                    stage_D2C_pair(gi - 2, gi - 1)
                    stage_D3(gi - 2)
                    stage_D3(gi - 1)
                if gi in (4, 8):
                    load_unit(gi // 4 + 1, dtsI)  # prefetch next unit
            stage_D1B(NG - 1)
            stage_D2C_pair(NG - 2, NG - 1)
            stage_D3(NG - 2)
            stage_D3(NG - 1)

    nc.compile()
    return nc


def _get_program():
    if "nc" not in _CACHE:
        _CACHE["nc"] = _build()
    return _CACHE["nc"]


def _host_prep(x, w1, b1, ln_w, ln_b, w2, b2):
    bf = ml_dtypes.bfloat16
    x = np.ascontiguousarray(np.asarray(x, dtype=np.float32))
    w1 = np.asarray(w1, dtype=np.float32)
    xm = x.reshape(N * C, T, L, H, W)
    # pad W by 1 on each side only (dw-planes); pad H rows on the fly
    xpadw = np.zeros((N * C, T, L, H, W + 2), np.float32)
    xpadw[:, :, :, :, 1:W + 1] = xm

    def slice_block(t, l, half):
        """[P, 3, NROW, W] bf16: 3 dw-planes of input rows for one half."""
        blk = np.zeros((P, 3, NROW, W), np.float32)
        if 0 <= t < T and 0 <= l < L:
            r0 = half * 16 - 1
            rows = xpadw[:, t, l]          # [P, H, W+2]
            lo, hi = max(r0, 0), min(r0 + NROW, H)
            for j in range(3):             # dw = j-1 -> col offset j
                blk[:, j, lo - r0: hi - r0, :] = rows[:, lo:hi, j: j + W]
        return blk.astype(bf)

    # per-core xin + w1c
    w1t = w1.transpose(1, 2, 3, 4, 5, 0)   # [Cin, dt, dl, dh, dw, Cout]
    xins, w1cs = [], []
    for k in range(8):
        xin = np.zeros((NU, 3, 5, P, 3, NROW, W), bf, order="C")
        rows_e = np.zeros((NW_EDGE, C, C), np.float32)
        rows_i = np.zeros((NW_INT, C, C), np.float32)
        for u, (t, half, lr) in enumerate(UNITS[k]):
            lvals, dts, flip = _unit_geometry(t, half, lr)
            for d, dt in enumerate(dts):
                for s in range(5):
                    xin[u, d, s] = slice_block(t + dt, lvals[s], half)
            # weight rows for this unit's class (edge u==0, interior shared)
            tgt = rows_e if u == 0 else rows_i
            for dls, dl in enumerate((0, 1, -1)):
                for d, dt in enumerate(dts):
                    for dh in range(3):
                        for dw in range(3):
                            r = dls * 9 * len(dts) + d * 9 + dh * 3 + dw
                            tgt[r] = w1t[:, dt + 1, flip * dl + 1,
                                         dh, dw, :]
        wfull = np.concatenate([rows_e, rows_i], axis=0)  # [NW, C, C]
        w1bd = np.zeros((NW, P, P), np.float32)
        w1bd[:, :C, :C] = wfull
        w1bd[:, C:, C:] = wfull
        xins.append(np.ascontiguousarray(xin.reshape(NU, 3, 5, P, 3, FLAT)))
        w1cs.append(np.ascontiguousarray(
            w1bd.transpose(1, 0, 2)).astype(bf))

    w2t = np.asarray(w2, dtype=np.float32).reshape(C, C).T
    w2bd = np.zeros((P, P), np.float32)
    w2bd[:C, :C] = w2t
    w2bd[C:, C:] = w2t
    onesbc = np.zeros((P, P), np.float32)
    onesbc[:C, :C] = 1.0 / C
    onesbc[C:, C:] = 1.0 / C
    params = np.zeros((P, 5), np.float32)
    params[:, 0] = np.tile(np.asarray(b1, dtype=np.float32), 2)
    params[:, 1] = np.tile(np.asarray(ln_w, dtype=np.float32), 2)
    params[:, 2] = np.tile(np.asarray(ln_b, dtype=np.float32), 2)
    params[:, 3] = np.tile(np.asarray(b2, dtype=np.float32), 2)
    params[:, 4] = EPS
    return xins, w1cs, w2bd.astype(bf), onesbc.astype(bf), params


def kernel(x, w1, b1, ln_w, ln_b, w2, b2):
    global LAST_RESULTS
    xins, w1cs, w2bd, onesbc, params = _host_prep(
        x, w1, b1, ln_w, ln_b, w2, b2)
    nc = _get_program()
    in_maps = [
        {"xin": xins[k], "w1c": w1cs[k], "w2bd": w2bd, "onesbc": onesbc,
         "params": params}
        for k in range(8)
    ]
    res = bass_utils.run_bass_kernel_spmd(
        nc, in_maps, core_ids=list(range(8)), trace=TRACE)
    LAST_RESULTS = res
    out = np.zeros((N, C, T, L, H, W), np.float32)
    for k in range(8):
        o = res.results[k]["out"]          # [P, NG, 512]
        for u, (t, half, lr) in enumerate(UNITS[k]):
            lvals, _, _ = _unit_geometry(t, half, lr)
            for g in range(4):
                blk = o[:, u * 4 + g].reshape(N, C, 16, W)
                out[:, :, t, lvals[g], 16 * half:16 * half + 16, :] = blk
    return np.ascontiguousarray(out)


# revision 11
# speedup vs baseline: 1.1941x; 1.0002x over previous
"""Trainium2 Bass kernel for a 4D ConvBlock (conv3^4 -> LN -> GELU -> 1x1 conv -> residual).

Strategy (8 NeuronCores, class-balanced work sharding):
  - Work unit = (t, H-half, L-range-of-4). 32 units globally; each core gets
    exactly 1 edge-t unit + 3 interior-t units = 1089 conv matmuls/core
    (perfectly balanced; a plain T-shard gives interior cores 1188 while
    edge cores idle 1/3 of their taps on zero halos).
  - Units with the upper L-range are processed L-flipped (host reorders
    slices and negates dl in the weight layout) so every unit uniformly has
    its L-edge at position 0. Same compiled program on all 8 cores; all
    per-core differences live in host-prepared input tensors.
  - Partition layout: 128 SBUF partitions = (sample n)*64 + channel c.
    conv1 = accumulating PE matmuls with BLOCK-DIAGONAL [128,128] weights so
    both batch samples ride one matmul.
  - Each input slice is stored as 3 pre-shifted dw-planes of [18,32] rows so
    every tap's matmul rhs is a single flat contiguous 512-element slice
    (contiguous moving operands stream ~6% faster than strided windows).
  - All matmul operands bf16 (PE full rate; fp8 DoubleRow measured NO
    faster per matmul wall-time, and plain fp8 fails the accuracy gate).
  - Channel-wise LayerNorm stats via block-diag ones matmuls (mean and
    mean-of-squares broadcast in one PE op each); exact-erf GELU on ACT.
  - Emission is software-pipelined A(g);A(g+1);B(g);A(g+2);B(g+1);C(g);...
    so the PE never stalls on the DVE/ACT LayerNorm chain between groups.
    PSUM budget: 2 acc + 4 stat + 2 out = exactly 8 banks.
"""
import os
import sys

os.environ.setdefault("MYCRO_LOCAL_CACHE", "1")
for _p in ("/opt/trn_rl_repo",):
    if os.path.isdir(_p) and _p not in sys.path:
        sys.path.insert(0, _p)

import numpy as np
import ml_dtypes

import concourse.bass as bass
import concourse.tile as tile
from concourse import bacc, mybir
from concourse import bass_utils

TRACE = os.environ.get("KERNEL_TRACE", "0") == "1"
MM_DTYPE = "bfloat16"  # test harness prints this

N, C, T, L, H, W = 2, 64, 8, 8, 32, 32
P = 128
EPS = 1e-5
NU = 4          # units per core
NG = 16         # acc groups per core (4 l-positions per unit)
NROW = 18       # input rows per half-slice (16 + dh halo)
FLAT = NROW * W # 576
NW_EDGE = 54    # edge-unit weight rows
NW_INT = 81     # interior-unit weight rows (shared by units 1..3)
NW = NW_EDGE + NW_INT  # 135

_CACHE = {}
LAST_RESULTS = None

# ---------------------------------------------------------------------------
# Unit table (global, fixed): unit = (t, half, lr). Edge units (t in {0,7})
# go one per core; interior units three per core.
_EDGE_UNITS = [(t, h, r) for t in (0, 7) for h in (0, 1) for r in (0, 1)]
# interior units ordered so each core's triple shares one lr (the three
# interior units of a core share one weight-row set, whose dl layout
# depends on the unit's l-flip)
_INT_UNITS = ([(t, h, 0) for t in range(1, 7) for h in (0, 1)] +
              [(t, h, 1) for t in range(1, 7) for h in (0, 1)])
UNITS = [[_EDGE_UNITS[k]] + _INT_UNITS[3 * k: 3 * k + 3] for k in range(8)]


def _unit_geometry(t, half, lr):
    """Returns (lvals, dt_list) for a unit. lvals[0] is the L-edge.
    dt_list[0] is always dt=0 (center)."""
    if lr == 0:
        lvals = [0, 1, 2, 3, 4]        # positions 0..3 + halo at index 4
        flip = 1
    else:
        lvals = [7, 6, 5, 4, 3]
        flip = -1
    if t == 0:
        dts = [0, 1]
    elif t == T - 1:
        dts = [0, -1]
    else:
        dts = [0, -1, 1]
    return lvals, dts, flip


def _build():
    f32 = mybir.dt.float32
    bf16 = mybir.dt.bfloat16
    AF = mybir.ActivationFunctionType

    nc = bacc.Bacc("TRN2", target_bir_lowering=False, debug=False,
                   enable_asserts=False, num_devices=8)
    # xin[u, d, s] = one input slice: 3 dw-planes x 18 rows x 32 cols
    xin = nc.dram_tensor("xin", [NU, 3, 5, P, 3, FLAT], bf16,
                         kind="ExternalInput").ap()
    w1c = nc.dram_tensor("w1c", [P, NW, P], bf16, kind="ExternalInput").ap()
    w2bd = nc.dram_tensor("w2bd", [P, P], bf16, kind="ExternalInput").ap()
    onesbc = nc.dram_tensor("onesbc", [P, P], bf16, kind="ExternalInput").ap()
    params = nc.dram_tensor("params", [P, 5], f32, kind="ExternalInput").ap()
    out = nc.dram_tensor("out", [P, NG, 512], f32, kind="ExternalOutput").ap()

    # weight row layout (must match host):
    #   edge rows:     (dls<2 first) idx = dls*18 + d*9 + dh*3 + dw   d in 0..1
    #   interior rows: 54 + dls*27 + d*9 + dh*3 + dw                  d in 0..2
    def wrow(u, dls, d, dh, dw):
        if u == 0:
            return dls * 18 + d * 9 + dh * 3 + dw
        return NW_EDGE + dls * 27 + d * 9 + dh * 3 + dw

    with tile.TileContext(nc) as tc:
        with (
            tc.tile_pool(name="wpool", bufs=1) as wpool,
            tc.tile_pool(name="xpool", bufs=2) as xpool,
            tc.tile_pool(name="work", bufs=2) as work,
            tc.tile_pool(name="respool", bufs=4) as respool,
            tc.tile_pool(name="ps_acc", bufs=2, space=bass.MemorySpace.PSUM) as ps_acc,
            tc.tile_pool(name="ps_bc", bufs=2, space=bass.MemorySpace.PSUM) as ps_bc,
            tc.tile_pool(name="ps_out", bufs=2, space=bass.MemorySpace.PSUM) as ps_out,
        ):
            # --- weights, chunked in consumption order -------------------
            w1sb = wpool.tile([P, NW, P], bf16, name="w1sb", tag="w1sb")
            wchunks = [(0, 9), (9, 18), (18, 36), (36, 54), (54, 108), (108, 135)]
            wloaded = 0

            def emit_wchunk():
                nonlocal wloaded
                a, b = wchunks[wloaded]
                nc.sync.dma_start(w1sb[:, a:b, :], w1c[:, a:b, :])
                wloaded += 1

            xs = {}

            def load_one(u, d, s):
                xt = xpool.tile([P, 3, FLAT], bf16,
                                name=f"x{u}_{d}_{s}", tag=f"x{d}_{s}")
                nc.sync.dma_start(xt[:], xin[u, d, s])
                xs[(u, d, s)] = xt

            def load_unit(u, dts, only_s=None):
                for s in (range(5) if only_s is None else only_s):
                    for d in range(len(dts)):
                        if (u, d, s) not in xs:
                            load_one(u, d, s)

            psb = wpool.tile([P, 5], f32, name="psb", tag="psb")
            w2sb = wpool.tile([P, P], bf16, name="w2sb", tag="w2sb")
            onsb = wpool.tile([P, P], bf16, name="onsb", tag="onsb")

            def emit_smalls():
                nc.sync.dma_start(psb[:], params[:])
                nc.sync.dma_start(w2sb[:], w2bd[:])
                nc.sync.dma_start(onsb[:], onesbc[:])

            b1_ap = psb[:, 0:1]
            lnw_ap = psb[:, 1:2]
            lnb_ap = psb[:, 2:3]
            b2_ap = psb[:, 3:4]
            eps_ap = psb[:, 4:5]

            state = {}

            def stage_A(gi):
                u, g = divmod(gi, 4)
                ndt = 2 if u == 0 else 3
                acc = ps_acc.tile([P, 512], f32, name=f"acc{gi}", tag="acc")
                # dl-slots: 0 -> slice g, 1 -> slice g+1, 2 -> slice g-1
                dls_list = [0, 1] if g == 0 else [0, 1, 2]
                slices = {0: g, 1: g + 1, 2: g - 1}
                n = sum(1 for _ in dls_list) * ndt * 9
                i = 0
                for dls in dls_list:
                    for d in range(ndt):
                        xt = xs[(u, d, slices[dls])]
                        for dh in range(3):
                            base = dh * W
                            for dw in range(3):
                                nc.tensor.matmul(
                                    acc[:],
                                    w1sb[:, wrow(u, dls, d, dh, dw), :],
                                    xt[:, dw, base: base + 512],
                                    start=(i == 0), stop=(i == n - 1))
                                i += 1
                state[("acc", gi)] = acc

            def stage_D1B(gi):
                acc = state.pop(("acc", gi))
                h = work.tile([P, 512], bf16, name=f"h{gi}", tag="h")
                nc.vector.tensor_scalar_add(h[:], acc[:], b1_ap)
                sq = work.tile([P, 512], bf16, name=f"sq{gi}", tag="sq")
                nc.vector.tensor_mul(sq[:], h[:], h[:])
                bc_mu = ps_bc.tile([P, 512], f32, name=f"bcmu{gi}", tag="bc_mu")
                nc.tensor.matmul(bc_mu[:], onsb[:], h[:])
                bc_e2 = ps_bc.tile([P, 512], f32, name=f"bce2{gi}", tag="bc_e2")
                nc.tensor.matmul(bc_e2[:], onsb[:], sq[:])
                u, g = divmod(gi, 4)
                res = respool.tile([P, 512], bf16, name=f"res{gi}", tag="res")
                nc.vector.tensor_copy(res[:], xs[(u, 0, g)][:, 1, W: W + 512])
                state[("h", gi)] = h
                state[("bc", gi)] = (bc_mu, bc_e2)
                state[("res", gi)] = res

            def stage_D2C_single(ga):
                h = state.pop(("h", ga))
                bc_mu, bc_e2 = state.pop(("bc", ga))
                mu_sbf = work.tile([P, 512], f32, name=f"mu{ga}", tag="mu_sbf")
                nc.vector.tensor_copy(mu_sbf[:], bc_mu[:])
                mu2 = work.tile([P, 512], f32, name=f"mu2{ga}", tag="mu2")
                nc.vector.tensor_mul(mu2[:], mu_sbf[:], mu_sbf[:])
                var = work.tile([P, 512], f32, name=f"var{ga}", tag="var")
                nc.vector.tensor_sub(var[:], bc_e2[:], mu2[:])
                rstd = work.tile([P, 512], f32, name=f"rstd{ga}", tag="rstd")
                nc.scalar.activation(rstd[:], var[:], AF.Abs_reciprocal_sqrt,
                                     bias=eps_ap, scale=1.0)
                t1 = work.tile([P, 512], f32, name=f"t1{ga}", tag="t1")
                nc.vector.tensor_sub(t1[:], h[:], mu_sbf[:])
                t2 = work.tile([P, 512], f32, name=f"t2{ga}", tag="t2")
                nc.vector.tensor_mul(t2[:], t1[:], rstd[:])
                g8 = work.tile([P, 512], bf16, name=f"g8{ga}", tag="g8")
                nc.scalar.activation(g8[:], t2[:], AF.Gelu,
                                     bias=lnb_ap, scale=lnw_ap)
                ps2 = ps_out.tile([P, 512], f32, name=f"ps2{ga}", tag="ps2")
                nc.tensor.matmul(ps2[:], w2sb[:], g8[:])
                state[("ps2", ga)] = ps2

            def stage_D2C_pair(ga, gb):
                # var chains for both groups, then rstd+rstd / gelu+gelu so
                # the ACT engine swaps its function table 2x per pair, not 4x
                hs, mus, vars_, rstds = {}, {}, {}, {}
                for gi in (ga, gb):
                    h = state.pop(("h", gi))
                    bc_mu, bc_e2 = state.pop(("bc", gi))
                    mu_sbf = work.tile([P, 512], f32, name=f"mu{gi}",
                                       tag="mu_sbf")
                    nc.vector.tensor_copy(mu_sbf[:], bc_mu[:])
                    mu2 = work.tile([P, 512], f32, name=f"mu2{gi}", tag="mu2")
                    nc.vector.tensor_mul(mu2[:], mu_sbf[:], mu_sbf[:])
                    var = work.tile([P, 512], f32, name=f"var{gi}", tag="var")
                    nc.vector.tensor_sub(var[:], bc_e2[:], mu2[:])
                    hs[gi], mus[gi], vars_[gi] = h, mu_sbf, var
                rstd_is = []
                for gi in (ga, gb):
                    rstd = work.tile([P, 512], f32, name=f"rstd{gi}",
                                     tag="rstd")
                    rstd_is.append(nc.scalar.activation(
                        rstd[:], vars_[gi][:], AF.Abs_reciprocal_sqrt,
                        bias=eps_ap, scale=1.0))
                    rstds[gi] = rstd
                t2s = {}
                for gi in (ga, gb):
                    t1 = work.tile([P, 512], f32, name=f"t1{gi}", tag="t1")
                    nc.vector.tensor_sub(t1[:], hs[gi][:], mus[gi][:])
                    t2 = work.tile([P, 512], f32, name=f"t2{gi}", tag="t2")
                    nc.vector.tensor_mul(t2[:], t1[:], rstds[gi][:])
                    t2s[gi] = t2
                first_gelu = True
                for gi in (ga, gb):
                    g8 = work.tile([P, 512], bf16, name=f"g8{gi}", tag="g8")
                    gelu_i = nc.scalar.activation(g8[:], t2s[gi][:], AF.Gelu,
                                                  bias=lnb_ap, scale=lnw_ap)
                    if first_gelu:
                        tile.add_dep_helper(
                            gelu_i.ins, rstd_is[1].ins, sync=True,
                            reason="batch ACT tables: absr,absr,gelu,gelu")
                        first_gelu = False
                    ps2 = ps_out.tile([P, 512], f32, name=f"ps2{gi}",
                                      tag="ps2")
                    nc.tensor.matmul(ps2[:], w2sb[:], g8[:])
                    state[("ps2", gi)] = ps2

            def stage_D3(gi):
                u, g = divmod(gi, 4)
                ps2 = state.pop(("ps2", gi))
                o1 = work.tile([P, 512], f32, name=f"o1{gi}", tag="o1")
                nc.vector.tensor_scalar_add(o1[:], ps2[:], b2_ap)
                osb = work.tile([P, 512], f32, name=f"osb{gi}", tag="osb")
                res = state.pop(("res", gi))
                nc.vector.tensor_add(osb[:], o1[:], res[:])
                nc.sync.dma_start(out[:, gi, :], osb[:])

            # ----- emission schedule -------------------------------------
            # loads: unit0 g0-critical slices, weights chunk0, smalls, rest
            u0 = UNITS[0][0]  # geometry identical across cores
            _, dts0, _ = _unit_geometry(*u0)
            # critical path for mm #0: slice (d=0,s=0) + weight rows 0..9
            load_one(0, 0, 0)
            emit_wchunk()            # rows 0..9    (g0, dls=0, d=0)
            load_one(0, 1, 0)
            emit_wchunk()            # rows 9..18   (g0, dls=0, d=1)
            load_unit(0, dts0, only_s=(1,))
            emit_wchunk()            # rows 18..36  (g0, dls=1 taps)
            emit_smalls()
            load_unit(0, dts0, only_s=(2, 3, 4))
            emit_wchunk()            # rows 36..54
            _, dtsI, _ = _unit_geometry(*UNITS[0][1])
            load_unit(1, dtsI, only_s=(0, 1))
            emit_wchunk()            # rows 54..108
            load_unit(1, dtsI, only_s=(2, 3, 4))
            emit_wchunk()            # rows 108..135

            for gi in range(NG):
                stage_A(gi)
                if gi >= 1:
                    stage_D1B(gi - 1)
                if gi == 2:
                    stage_D2C_single(0)
                    stage_D3(0)
                if gi >= 3 and gi % 2 == 1:
                    stage_D2C_pair(gi - 2, gi - 1)
                    stage_D3(gi - 2)
                    stage_D3(gi - 1)
                if gi in (4, 8):
                    load_unit(gi // 4 + 1, dtsI)  # prefetch next unit
            stage_D1B(NG - 1)
            stage_D2C_single(NG - 1)
            stage_D3(NG - 1)

    nc.compile()
    return nc


def _get_program():
    if "nc" not in _CACHE:
        _CACHE["nc"] = _build()
    return _CACHE["nc"]


def _host_prep(x, w1, b1, ln_w, ln_b, w2, b2):
    bf = ml_dtypes.bfloat16
    x = np.ascontiguousarray(np.asarray(x, dtype=np.float32))
    w1 = np.asarray(w1, dtype=np.float32)
    xm = x.reshape(N * C, T, L, H, W)
    # pad W by 1 on each side only (dw-planes); pad H rows on the fly
    xpadw = np.zeros((N * C, T, L, H, W + 2), np.float32)
    xpadw[:, :, :, :, 1:W + 1] = xm

    def slice_block(t, l, half):
        """[P, 3, NROW, W] bf16: 3 dw-planes of input rows for one half."""
        blk = np.zeros((P, 3, NROW, W), np.float32)
        if 0 <= t < T and 0 <= l < L:
            r0 = half * 16 - 1
            rows = xpadw[:, t, l]          # [P, H, W+2]
            lo, hi = max(r0, 0), min(r0 + NROW, H)
            for j in range(3):             # dw = j-1 -> col offset j
                blk[:, j, lo - r0: hi - r0, :] = rows[:, lo:hi, j: j + W]
        return blk.astype(bf)

    # per-core xin + w1c
    w1t = w1.transpose(1, 2, 3, 4, 5, 0)   # [Cin, dt, dl, dh, dw, Cout]
    xins, w1cs = [], []
    for k in range(8):
        xin = np.zeros((NU, 3, 5, P, 3, NROW, W), bf, order="C")
        rows_e = np.zeros((NW_EDGE, C, C), np.float32)
        rows_i = np.zeros((NW_INT, C, C), np.float32)
        for u, (t, half, lr) in enumerate(UNITS[k]):
            lvals, dts, flip = _unit_geometry(t, half, lr)
            for d, dt in enumerate(dts):
                for s in range(5):
                    xin[u, d, s] = slice_block(t + dt, lvals[s], half)
            # weight rows for this unit's class (edge u==0, interior shared)
            tgt = rows_e if u == 0 else rows_i
            for dls, dl in enumerate((0, 1, -1)):
                for d, dt in enumerate(dts):
                    for dh in range(3):
                        for dw in range(3):
                            r = dls * 9 * len(dts) + d * 9 + dh * 3 + dw
                            tgt[r] = w1t[:, dt + 1, flip * dl + 1,
                                         dh, dw, :]
        wfull = np.concatenate([rows_e, rows_i], axis=0)  # [NW, C, C]
        w1bd = np.zeros((NW, P, P), np.float32)
        w1bd[:, :C, :C] = wfull
        w1bd[:, C:, C:] = wfull
        xins.append(np.ascontiguousarray(xin.reshape(NU, 3, 5, P, 3, FLAT)))
        w1cs.append(np.ascontiguousarray(
            w1bd.transpose(1, 0, 2)).astype(bf))

    w2t = np.asarray(w2, dtype=np.float32).reshape(C, C).T
    w2bd = np.zeros((P, P), np.float32)
    w2bd[:C, :C] = w2t
    w2bd[C:, C:] = w2t
    onesbc = np.zeros((P, P), np.float32)
    onesbc[:C, :C] = 1.0 / C
    onesbc[C:, C:] = 1.0 / C
    params = np.zeros((P, 5), np.float32)
    params[:, 0] = np.tile(np.asarray(b1, dtype=np.float32), 2)
    params[:, 1] = np.tile(np.asarray(ln_w, dtype=np.float32), 2)
    params[:, 2] = np.tile(np.asarray(ln_b, dtype=np.float32), 2)
    params[:, 3] = np.tile(np.asarray(b2, dtype=np.float32), 2)
    params[:, 4] = EPS
    return xins, w1cs, w2bd.astype(bf), onesbc.astype(bf), params


def kernel(x, w1, b1, ln_w, ln_b, w2, b2):
    global LAST_RESULTS
    xins, w1cs, w2bd, onesbc, params = _host_prep(
        x, w1, b1, ln_w, ln_b, w2, b2)
    nc = _get_program()
    in_maps = [
        {"xin": xins[k], "w1c": w1cs[k], "w2bd": w2bd, "onesbc": onesbc,
         "params": params}
        for k in range(8)
    ]
    res = bass_utils.run_bass_kernel_spmd(
        nc, in_maps, core_ids=list(range(8)), trace=TRACE)
    LAST_RESULTS = res
    out = np.zeros((N, C, T, L, H, W), np.float32)
    for k in range(8):
        o = res.results[k]["out"]          # [P, NG, 512]
        for u, (t, half, lr) in enumerate(UNITS[k]):
            lvals, _, _ = _unit_geometry(t, half, lr)
            for g in range(4):
                blk = o[:, u * 4 + g].reshape(N, C, 16, W)
                out[:, :, t, lvals[g], 16 * half:16 * half + 16, :] = blk
    return np.ascontiguousarray(out)


# revision 13
# speedup vs baseline: 1.2055x; 1.0095x over previous
"""Trainium2 Bass kernel for a 4D ConvBlock (conv3^4 -> LN -> GELU -> 1x1 conv -> residual).

Strategy (8 NeuronCores, class-balanced work sharding):
  - Work unit = (t, H-half, L-range-of-4). 32 units globally; each core gets
    exactly 1 edge-t unit + 3 interior-t units = 1089 conv matmuls/core
    (perfectly balanced; a plain T-shard gives interior cores 1188 while
    edge cores idle 1/3 of their taps on zero halos).
  - Units with the upper L-range are processed L-flipped (host reorders
    slices and negates dl in the weight layout) so every unit uniformly has
    its L-edge at position 0. Same compiled program on all 8 cores; all
    per-core differences live in host-prepared input tensors.
  - Partition layout: 128 SBUF partitions = (sample n)*64 + channel c.
    conv1 = accumulating PE matmuls with BLOCK-DIAGONAL [128,128] weights so
    both batch samples ride one matmul.
  - Each input slice is stored as 3 pre-shifted dw-planes of [18,32] rows so
    every tap's matmul rhs is a single flat contiguous 512-element slice
    (contiguous moving operands stream ~6% faster than strided windows).
  - All matmul operands bf16 (PE full rate; fp8 DoubleRow measured NO
    faster per matmul wall-time, and plain fp8 fails the accuracy gate).
  - Channel-wise LayerNorm stats via block-diag ones matmuls (mean and
    mean-of-squares broadcast in one PE op each); exact-erf GELU on ACT.
  - Emission is software-pipelined A(g);A(g+1);B(g);A(g+2);B(g+1);C(g);...
    so the PE never stalls on the DVE/ACT LayerNorm chain between groups.
    PSUM budget: 2 acc + 4 stat + 2 out = exactly 8 banks.
"""
import os
import sys

os.environ.setdefault("MYCRO_LOCAL_CACHE", "1")
for _p in ("/opt/trn_rl_repo",):
    if os.path.isdir(_p) and _p not in sys.path:
        sys.path.insert(0, _p)

import numpy as np
import ml_dtypes

import concourse.bass as bass
import concourse.tile as tile
from concourse import bacc, mybir
from concourse import bass_utils

TRACE = os.environ.get("KERNEL_TRACE", "0") == "1"
MM_DTYPE = "bfloat16"  # test harness prints this

N, C, T, L, H, W = 2, 64, 8, 8, 32, 32
P = 128
EPS = 1e-5
NU = 4          # units per core
NG = 16         # acc groups per core (4 l-positions per unit)
NROW = 18       # input rows per half-slice (16 + dh halo)
FLAT = NROW * W # 576
NW_EDGE = 54    # edge-unit weight rows
NW_INT = 81     # interior-unit weight rows (shared by units 1..3)
NW = NW_EDGE + NW_INT  # 135

_CACHE = {}
LAST_RESULTS = None

# ---------------------------------------------------------------------------
# Unit table (global, fixed): unit = (t, half, lr). Edge units (t in {0,7})
# go one per core; interior units three per core.
_EDGE_UNITS = [(t, h, r) for t in (0, 7) for h in (0, 1) for r in (0, 1)]
# interior units ordered so each core's triple shares one lr (the three
# interior units of a core share one weight-row set, whose dl layout
# depends on the unit's l-flip)
_INT_UNITS = ([(t, h, 0) for t in range(1, 7) for h in (0, 1)] +
              [(t, h, 1) for t in range(1, 7) for h in (0, 1)])
UNITS = [[_EDGE_UNITS[k]] + _INT_UNITS[3 * k: 3 * k + 3] for k in range(8)]


def _unit_geometry(t, half, lr):
    """Returns (lvals, dt_list) for a unit. lvals[0] is the L-edge.
    dt_list[0] is always dt=0 (center)."""
    if lr == 0:
        lvals = [0, 1, 2, 3, 4]        # positions 0..3 + halo at index 4
        flip = 1
    else:
        lvals = [7, 6, 5, 4, 3]
        flip = -1
    if t == 0:
        dts = [0, 1]
    elif t == T - 1:
        dts = [0, -1]
    else:
        dts = [0, -1, 1]
    return lvals, dts, flip


def _build():
    f32 = mybir.dt.float32
    bf16 = mybir.dt.bfloat16
    AF = mybir.ActivationFunctionType

    nc = bacc.Bacc("TRN2", target_bir_lowering=False, debug=False,
                   enable_asserts=False, num_devices=8)
    # xin[u, d, s] = one input slice: 3 dw-planes x 18 rows x 32 cols
    xin = nc.dram_tensor("xin", [NU, 3, 5, P, 3, FLAT], bf16,
                         kind="ExternalInput").ap()
    w1c = nc.dram_tensor("w1c", [P, NW, P], bf16, kind="ExternalInput").ap()
    w2bd = nc.dram_tensor("w2bd", [P, P], bf16, kind="ExternalInput").ap()
    onesbc = nc.dram_tensor("onesbc", [P, P], bf16, kind="ExternalInput").ap()
    params = nc.dram_tensor("params", [P, 5], f32, kind="ExternalInput").ap()
    out = nc.dram_tensor("out", [P, NG, 512], f32, kind="ExternalOutput").ap()

    # weight row layout (must match host):
    #   edge rows:     (dls<2 first) idx = dls*18 + d*9 + dh*3 + dw   d in 0..1
    #   interior rows: 54 + dls*27 + d*9 + dh*3 + dw                  d in 0..2
    def wrow(u, dls, d, dh, dw):
        if u == 0:
            return dls * 18 + d * 9 + dh * 3 + dw
        return NW_EDGE + dls * 27 + d * 9 + dh * 3 + dw

    with tile.TileContext(nc) as tc:
        with (
            tc.tile_pool(name="wpool", bufs=1) as wpool,
            tc.tile_pool(name="xpool", bufs=2) as xpool,
            tc.tile_pool(name="work", bufs=2) as work,
            tc.tile_pool(name="respool", bufs=4) as respool,
            tc.tile_pool(name="ps_acc", bufs=2, space=bass.MemorySpace.PSUM) as ps_acc,
            tc.tile_pool(name="ps_bc", bufs=2, space=bass.MemorySpace.PSUM) as ps_bc,
            tc.tile_pool(name="ps_out", bufs=2, space=bass.MemorySpace.PSUM) as ps_out,
        ):
            # --- weights, chunked in consumption order -------------------
            w1sb = wpool.tile([P, NW, P], bf16, name="w1sb", tag="w1sb")
            wchunks = [(0, 9), (9, 18), (18, 36), (36, 54), (54, 108), (108, 135)]
            wloaded = 0

            def emit_wchunk():
                nonlocal wloaded
                a, b = wchunks[wloaded]
                nc.sync.dma_start(w1sb[:, a:b, :], w1c[:, a:b, :])
                wloaded += 1

            xs = {}

            def load_one(u, d, s):
                xt = xpool.tile([P, 3, FLAT], bf16,
                                name=f"x{u}_{d}_{s}", tag=f"x{d}_{s}")
                nc.sync.dma_start(xt[:], xin[u, d, s])
                xs[(u, d, s)] = xt

            def load_unit(u, dts, only_s=None):
                for s in (range(5) if only_s is None else only_s):
                    for d in range(len(dts)):
                        if (u, d, s) not in xs:
                            load_one(u, d, s)

            psb = wpool.tile([P, 5], f32, name="psb", tag="psb")
            w2sb = wpool.tile([P, P], bf16, name="w2sb", tag="w2sb")
            onsb = wpool.tile([P, P], bf16, name="onsb", tag="onsb")

            def emit_smalls():
                nc.sync.dma_start(psb[:], params[:])
                nc.sync.dma_start(w2sb[:], w2bd[:])
                nc.sync.dma_start(onsb[:], onesbc[:])

            b1_ap = psb[:, 0:1]
            lnw_ap = psb[:, 1:2]
            lnb_ap = psb[:, 2:3]
            b2_ap = psb[:, 3:4]
            eps_ap = psb[:, 4:5]

            state = {}

            def stage_A(gi):
                u, g = divmod(gi, 4)
                ndt = 2 if u == 0 else 3
                acc = ps_acc.tile([P, 512], f32, name=f"acc{gi}", tag="acc")
                # dl-slots: 0 -> slice g, 1 -> slice g+1, 2 -> slice g-1
                dls_list = [0, 1] if g == 0 else [0, 1, 2]
                slices = {0: g, 1: g + 1, 2: g - 1}
                n = sum(1 for _ in dls_list) * ndt * 9
                i = 0
                for dls in dls_list:
                    for d in range(ndt):
                        xt = xs[(u, d, slices[dls])]
                        for dh in range(3):
                            base = dh * W
                            for dw in range(3):
                                nc.tensor.matmul(
                                    acc[:],
                                    w1sb[:, wrow(u, dls, d, dh, dw), :],
                                    xt[:, dw, base: base + 512],
                                    start=(i == 0), stop=(i == n - 1))
                                i += 1
                state[("acc", gi)] = acc

            def stage_D1B(gi):
                acc = state.pop(("acc", gi))
                h = work.tile([P, 512], bf16, name=f"h{gi}", tag="h")
                nc.vector.tensor_scalar_add(h[:], acc[:], b1_ap)
                sq = work.tile([P, 512], bf16, name=f"sq{gi}", tag="sq")
                nc.vector.tensor_mul(sq[:], h[:], h[:])
                bc_mu = ps_bc.tile([P, 512], f32, name=f"bcmu{gi}", tag="bc_mu")
                nc.tensor.matmul(bc_mu[:], onsb[:], h[:])
                bc_e2 = ps_bc.tile([P, 512], f32, name=f"bce2{gi}", tag="bc_e2")
                nc.tensor.matmul(bc_e2[:], onsb[:], sq[:])
                u, g = divmod(gi, 4)
                res = respool.tile([P, 512], bf16, name=f"res{gi}", tag="res")
                nc.vector.tensor_copy(res[:], xs[(u, 0, g)][:, 1, W: W + 512])
                state[("h", gi)] = h
                state[("bc", gi)] = (bc_mu, bc_e2)
                state[("res", gi)] = res

            def stage_D2C_single(ga):
                h = state.pop(("h", ga))
                bc_mu, bc_e2 = state.pop(("bc", ga))
                mu_sbf = work.tile([P, 512], f32, name=f"mu{ga}", tag="mu_sbf")
                nc.vector.tensor_copy(mu_sbf[:], bc_mu[:])
                mu2 = work.tile([P, 512], f32, name=f"mu2{ga}", tag="mu2")
                nc.vector.tensor_mul(mu2[:], mu_sbf[:], mu_sbf[:])
                var = work.tile([P, 512], f32, name=f"var{ga}", tag="var")
                nc.vector.tensor_sub(var[:], bc_e2[:], mu2[:])
                rstd = work.tile([P, 512], f32, name=f"rstd{ga}", tag="rstd")
                nc.scalar.activation(rstd[:], var[:], AF.Abs_reciprocal_sqrt,
                                     bias=eps_ap, scale=1.0)
                t1 = work.tile([P, 512], f32, name=f"t1{ga}", tag="t1")
                nc.vector.tensor_sub(t1[:], h[:], mu_sbf[:])
                t2 = work.tile([P, 512], f32, name=f"t2{ga}", tag="t2")
                nc.vector.tensor_mul(t2[:], t1[:], rstd[:])
                g8 = work.tile([P, 512], bf16, name=f"g8{ga}", tag="g8")
                nc.scalar.activation(g8[:], t2[:], AF.Gelu,
                                     bias=lnb_ap, scale=lnw_ap)
                ps2 = ps_out.tile([P, 512], f32, name=f"ps2{ga}", tag="ps2")
                nc.tensor.matmul(ps2[:], w2sb[:], g8[:])
                state[("ps2", ga)] = ps2

            def stage_D2C_pair(ga, gb):
                # var chains for both groups, then rstd+rstd / gelu+gelu so
                # the ACT engine swaps its function table 2x per pair, not 4x
                hs, mus, vars_, rstds = {}, {}, {}, {}
                for gi in (ga, gb):
                    h = state.pop(("h", gi))
                    bc_mu, bc_e2 = state.pop(("bc", gi))
                    mu_sbf = work.tile([P, 512], f32, name=f"mu{gi}",
                                       tag="mu_sbf")
                    nc.vector.tensor_copy(mu_sbf[:], bc_mu[:])
                    mu2 = work.tile([P, 512], f32, name=f"mu2{gi}", tag="mu2")
                    nc.vector.tensor_mul(mu2[:], mu_sbf[:], mu_sbf[:])
                    var = work.tile([P, 512], f32, name=f"var{gi}", tag="var")
                    nc.vector.tensor_sub(var[:], bc_e2[:], mu2[:])
                    hs[gi], mus[gi], vars_[gi] = h, mu_sbf, var
                rstd_is = []
                for gi in (ga, gb):
                    rstd = work.tile([P, 512], f32, name=f"rstd{gi}",
                                     tag="rstd")
                    rstd_is.append(nc.scalar.activation(
                        rstd[:], vars_[gi][:], AF.Abs_reciprocal_sqrt,
                        bias=eps_ap, scale=1.0))
                    rstds[gi] = rstd
                t2s = {}
                for gi in (ga, gb):
                    t1 = work.tile([P, 512], f32, name=f"t1{gi}", tag="t1")
                    nc.vector.tensor_sub(t1[:], hs[gi][:], mus[gi][:])
                    t2 = work.tile([P, 512], f32, name=f"t2{gi}", tag="t2")
                    nc.vector.tensor_mul(t2[:], t1[:], rstds[gi][:])
                    t2s[gi] = t2
                first_gelu = True
                for gi in (ga, gb):
                    g8 = work.tile([P, 512], bf16, name=f"g8{gi}", tag="g8")
                    gelu_i = nc.scalar.activation(g8[:], t2s[gi][:], AF.Gelu,
                                                  bias=lnb_ap, scale=lnw_ap)
                    if first_gelu:
                        tile.add_dep_helper(
                            gelu_i.ins, rstd_is[1].ins, sync=True,
                            reason="batch ACT tables: absr,absr,gelu,gelu")
                        first_gelu = False
                    ps2 = ps_out.tile([P, 512], f32, name=f"ps2{gi}",
                                      tag="ps2")
                    nc.tensor.matmul(ps2[:], w2sb[:], g8[:])
                    state[("ps2", gi)] = ps2

            def stage_D3(gi):
                u, g = divmod(gi, 4)
                ps2 = state.pop(("ps2", gi))
                o1 = work.tile([P, 512], f32, name=f"o1{gi}", tag="o1")
                nc.vector.tensor_scalar_add(o1[:], ps2[:], b2_ap)
                osb = work.tile([P, 512], f32, name=f"osb{gi}", tag="osb")
                res = state.pop(("res", gi))
                nc.vector.tensor_add(osb[:], o1[:], res[:])
                nc.sync.dma_start(out[:, gi, :], osb[:])

            # ----- emission schedule -------------------------------------
            # loads: unit0 g0-critical slices, weights chunk0, smalls, rest
            u0 = UNITS[0][0]  # geometry identical across cores
            _, dts0, _ = _unit_geometry(*u0)
            # critical path for mm #0: slice (d=0,s=0) + weight rows 0..9
            load_one(0, 0, 0)
            emit_wchunk()            # rows 0..9    (g0, dls=0, d=0)
            load_one(0, 1, 0)
            emit_wchunk()            # rows 9..18   (g0, dls=0, d=1)
            load_unit(0, dts0, only_s=(1,))
            emit_wchunk()            # rows 18..36  (g0, dls=1 taps)
            emit_smalls()
            load_unit(0, dts0, only_s=(2, 3, 4))
            emit_wchunk()            # rows 36..54
            _, dtsI, _ = _unit_geometry(*UNITS[0][1])
            load_unit(1, dtsI, only_s=(0, 1))
            emit_wchunk()            # rows 54..108
            load_unit(1, dtsI, only_s=(2, 3, 4))
            emit_wchunk()            # rows 108..135

            for gi in range(NG):
                if gi == NG - 1:
                    # final body: emit D1B(14) first so chains 13/14 run
                    # on DVE/ACT underneath A(15) instead of after it
                    stage_D1B(gi - 1)
                    stage_A(gi)
                    stage_D2C_pair(gi - 2, gi - 1)
                    stage_D3(gi - 2)
                    stage_D3(gi - 1)
                    continue
                stage_A(gi)
                if gi >= 1:
                    stage_D1B(gi - 1)
                if gi == 2:
                    stage_D2C_single(0)
                    stage_D3(0)
                if gi >= 3 and gi % 2 == 1 and gi < NG - 1:
                    stage_D2C_pair(gi - 2, gi - 1)
                    stage_D3(gi - 2)
                    stage_D3(gi - 1)
                if gi in (4, 8):
                    load_unit(gi // 4 + 1, dtsI)  # prefetch next unit
            stage_D1B(NG - 1)
            stage_D2C_single(NG - 1)
            stage_D3(NG - 1)

    nc.compile()
    return nc


def _get_program():
    if "nc" not in _CACHE:
        _CACHE["nc"] = _build()
    return _CACHE["nc"]


def _host_prep(x, w1, b1, ln_w, ln_b, w2, b2):
    bf = ml_dtypes.bfloat16
    x = np.ascontiguousarray(np.asarray(x, dtype=np.float32))
    w1 = np.asarray(w1, dtype=np.float32)
    xm = x.reshape(N * C, T, L, H, W)
    # pad W by 1 on each side only (dw-planes); pad H rows on the fly
    xpadw = np.zeros((N * C, T, L, H, W + 2), np.float32)
    xpadw[:, :, :, :, 1:W + 1] = xm

    def slice_block(t, l, half):
        """[P, 3, NROW, W] bf16: 3 dw-planes of input rows for one half."""
        blk = np.zeros((P, 3, NROW, W), np.float32)
        if 0 <= t < T and 0 <= l < L:
            r0 = half * 16 - 1
            rows = xpadw[:, t, l]          # [P, H, W+2]
            lo, hi = max(r0, 0), min(r0 + NROW, H)
            for j in range(3):             # dw = j-1 -> col offset j
                blk[:, j, lo - r0: hi - r0, :] = rows[:, lo:hi, j: j + W]
        return blk.astype(bf)

    # per-core xin + w1c
    w1t = w1.transpose(1, 2, 3, 4, 5, 0)   # [Cin, dt, dl, dh, dw, Cout]
    xins, w1cs = [], []
    for k in range(8):
        xin = np.zeros((NU, 3, 5, P, 3, NROW, W), bf, order="C")
        rows_e = np.zeros((NW_EDGE, C, C), np.float32)
        rows_i = np.zeros((NW_INT, C, C), np.float32)
        for u, (t, half, lr) in enumerate(UNITS[k]):
            lvals, dts, flip = _unit_geometry(t, half, lr)
            for d, dt in enumerate(dts):
                for s in range(5):
                    xin[u, d, s] = slice_block(t + dt, lvals[s], half)
            # weight rows for this unit's class (edge u==0, interior shared)
            tgt = rows_e if u == 0 else rows_i
            for dls, dl in enumerate((0, 1, -1)):
                for d, dt in enumerate(dts):
                    for dh in range(3):
                        for dw in range(3):
                            r = dls * 9 * len(dts) + d * 9 + dh * 3 + dw
                            tgt[r] = w1t[:, dt + 1, flip * dl + 1,
                                         dh, dw, :]
        wfull = np.concatenate([rows_e, rows_i], axis=0)  # [NW, C, C]
        w1bd = np.zeros((NW, P, P), np.float32)
        w1bd[:, :C, :C] = wfull
        w1bd[:, C:, C:] = wfull
        xins.append(np.ascontiguousarray(xin.reshape(NU, 3, 5, P, 3, FLAT)))
        w1cs.append(np.ascontiguousarray(
            w1bd.transpose(1, 0, 2)).astype(bf))

    w2t = np.asarray(w2, dtype=np.float32).reshape(C, C).T
    w2bd = np.zeros((P, P), np.float32)
    w2bd[:C, :C] = w2t
    w2bd[C:, C:] = w2t
    onesbc = np.zeros((P, P), np.float32)
    onesbc[:C, :C] = 1.0 / C
    onesbc[C:, C:] = 1.0 / C
    params = np.zeros((P, 5), np.float32)
    params[:, 0] = np.tile(np.asarray(b1, dtype=np.float32), 2)
    params[:, 1] = np.tile(np.asarray(ln_w, dtype=np.float32), 2)
    params[:, 2] = np.tile(np.asarray(ln_b, dtype=np.float32), 2)
    params[:, 3] = np.tile(np.asarray(b2, dtype=np.float32), 2)
    params[:, 4] = EPS
    return xins, w1cs, w2bd.astype(bf), onesbc.astype(bf), params


def kernel(x, w1, b1, ln_w, ln_b, w2, b2):
    global LAST_RESULTS
    xins, w1cs, w2bd, onesbc, params = _host_prep(
        x, w1, b1, ln_w, ln_b, w2, b2)
    nc = _get_program()
    in_maps = [
        {"xin": xins[k], "w1c": w1cs[k], "w2bd": w2bd, "onesbc": onesbc,
         "params": params}
        for k in range(8)
    ]
    res = bass_utils.run_bass_kernel_spmd(
        nc, in_maps, core_ids=list(range(8)), trace=TRACE)
    LAST_RESULTS = res
    out = np.zeros((N, C, T, L, H, W), np.float32)
    for k in range(8):
        o = res.results[k]["out"]          # [P, NG, 512]
        for u, (t, half, lr) in enumerate(UNITS[k]):
            lvals, _, _ = _unit_geometry(t, half, lr)
            for g in range(4):
                blk = o[:, u * 4 + g].reshape(N, C, 16, W)
                out[:, :, t, lvals[g], 16 * half:16 * half + 16, :] = blk
    return np.ascontiguousarray(out)
